# revision 39
# baseline (speedup 1.0000x reference)
"""Transformer block (B=4,T=2048,C=1024,H=16) on 8 trn2 cores, zero-communication.

Split: core c -> sequence b=c//2, token parity s=c%2. Each core computes the
full block output for its 1024 strided query tokens (positions s, s+2, ...),
recomputing LN1+K/V for the whole 2048-token context locally (no collectives).

Pipeline (v2): phases are software-pipelined to keep TensorE (and the HAM
clock) saturated end to end:
  front: LN1 tiles interleaved with kT / vA(oj=0) / qT(m=0) projections
  m=0:   attention q-cols 0..511 for all head-pairs; fillers = vA(oj=1),
         qT(m=1)
  m=1:   attention q-cols 512..1023; fillers = c_proj + residual for token
         tiles 0..3
  post:  c_proj qt 4..7, LN2 -> mT, then MLP in 4 f-quarters with
         double-buffered wf/wp streaming; proj accumulates into x2 in SBUF.

PSUM (8 banks): sc 2x[128,1024] (4) | av 2x[*,512] (2) | bc 1 (1) | fill 1 (1).
Softmax normalize: avp -> SBUF early-release copy, denom row broadcast by
matmul, reciprocal_approx_fast, tensor_mul.
"""
import sys

sys.path.insert(0, "/opt/trn_rl_repo")

import numpy as np
import ml_dtypes

import concourse.bass as bass
import concourse.mybir as mybir
import concourse.tile as tile
from concourse import bacc
from concourse.masks import make_identity

FP32 = mybir.dt.float32
BF16 = mybir.dt.bfloat16
AF = mybir.ActivationFunctionType
ALU = mybir.AluOpType

C = 1024
H = 16
HS = 64
FF = 4096
LN_EPS = 1e-5
P = 128


def build_nc(T=2048):
    own = T // 2          # query tokens per core
    NKV = T // P          # kv token tiles (16)
    QM = min(512, own)    # q-macro width
    NM = own // QM        # q macros (2)
    NQT = own // P        # own-token tiles (8)
    NCT = C // P          # feature tiles (8)
    NTM = T // 512        # kv 512-macros (4)

    nc = bacc.Bacc(None, target_bir_lowering=False, debug=False)

    xc = nc.dram_tensor("xc", [T, C], FP32, kind="ExternalInput")
    xq = nc.dram_tensor("xq", [own, C], FP32, kind="ExternalInput")
    wq = nc.dram_tensor("wq", [C, C], BF16, kind="ExternalInput")
    wk = nc.dram_tensor("wk", [C, C], BF16, kind="ExternalInput")
    wv = nc.dram_tensor("wv", [C, C], BF16, kind="ExternalInput")
    wc = nc.dram_tensor("wc", [C, C], BF16, kind="ExternalInput")
    wf = nc.dram_tensor("wf", [C, FF], BF16, kind="ExternalInput")
    wp = nc.dram_tensor("wp", [FF, C], BF16, kind="ExternalInput")
    msk = nc.dram_tensor("msk", [P, 2 * 64], BF16, kind="ExternalInput")
    yout = nc.dram_tensor("yout", [own, C], FP32, kind="ExternalOutput")

    with tile.TileContext(nc) as tc:
        import contextlib

        with contextlib.ExitStack() as ctx:
            const = ctx.enter_context(tc.tile_pool(name="const", bufs=1))
            xin = ctx.enter_context(tc.tile_pool(name="xin", bufs=2))
            lnp = ctx.enter_context(tc.tile_pool(name="lnp", bufs=3))
            nbp = ctx.enter_context(tc.tile_pool(name="nbp", bufs=2))
            g1 = ctx.enter_context(tc.tile_pool(name="g1", bufs=NCT))   # nT -> hq
            g2 = ctx.enter_context(tc.tile_pool(name="g2", bufs=NCT))   # kT -> wf/wp
            g3 = ctx.enter_context(tc.tile_pool(name="g3", bufs=NCT))   # nqT -> x2
            vap = ctx.enter_context(tc.tile_pool(name="vap", bufs=NKV))  # vA -> mT
            qwp = ctx.enter_context(tc.tile_pool(name="qwp", bufs=NCT))  # qT
            wcp = ctx.enter_context(tc.tile_pool(name="wcp", bufs=NCT))  # wc halves
            atp = ctx.enter_context(tc.tile_pool(name="atp", bufs=NCT))  # attnT
            exq = ctx.enter_context(tc.tile_pool(name="exq", bufs=3))    # ex + xr
            smp = ctx.enter_context(tc.tile_pool(name="smp", bufs=3))    # softmax
            wsp = ctx.enter_context(tc.tile_pool(name="wsp", bufs=8))    # wq/wk blk
            wvp = ctx.enter_context(tc.tile_pool(name="wvp", bufs=8))   # wv halves

            ps = ctx.enter_context(tc.tile_pool(name="ps", bufs=1, space="PSUM"))

            _psn = [0, 0]

            def ps_small(name, shape=(P, 512), dtype=FP32, in_attn=False):
                """Rotating one-bank psum tile.

                Outside attention: cycles av/bc/fill (ring of 4 incl. av's 2
                slots). Inside attention the av slots are HELD by the live
                softmax accumulators — allocating av there deadlocks the PE —
                so fillers cycle bc/fill only."""
                if in_attn:
                    tag = ("bc", "fill")[_psn[1] % 2]
                    _psn[1] += 1
                else:
                    tag = ("av", "bc", "fill")[_psn[0] % 3]
                    _psn[0] += 1
                bufs = {"av": 2, "bc": 1, "fill": 1}[tag]
                return ps.tile(list(shape), dtype, tag=tag, bufs=bufs, name=name)

            ident = const.tile([P, P], BF16)
            make_identity(nc, ident)
            ones65 = const.tile([65, 64], BF16)
            nc.vector.memset(ones65[64:65, :], 1.0)
            zrow = const.tile([1, 65], BF16)
            nc.vector.memset(zrow, 0.0)
            epst = const.tile([P, 1], FP32)
            nc.vector.memset(epst, LN_EPS)
            maskt = const.tile([P, 2 * 64], BF16)
            nc.sync.dma_start(maskt[:], msk[:])

            # PE warmup: HAM clock ramp while the first LN chunks stream in.
            for wi in range(48):
                wps = ps.tile([P, P], BF16, tag="sc", bufs=2, name=f"warm{wi}")
                nc.tensor.transpose(wps[:], ident[:], ident[:])

            _ceng = [0]

            def trans_copy(dst, src):
                """PSUM->SBUF [128,128] copy, alternated scalar/vector
                (GpSimd cannot read PSUM)."""
                k = _ceng[0] % 2
                _ceng[0] += 1
                if k == 0:
                    nc.scalar.activation(dst, src, AF.Copy)
                else:
                    nc.vector.tensor_copy(dst, src)

            def layer_norm_to_bf16(xt, out_bf, uid):
                """xt [128, C] f32 -> out_bf [128, C] bf16 (normalized).

                Stats on VectorE; the big apply on ScalarE via
                Identity(x*rs + (-mu*rs))."""
                stats = lnp.tile([P, 2, 6], FP32, tag="stats", name=f"st{uid}")
                nc.vector.bn_stats(stats[:, 0, :], xt[:, 0:512])
                nc.vector.bn_stats(stats[:, 1, :], xt[:, 512:1024])
                mv = lnp.tile([P, 2], FP32, tag="mv", name=f"mv{uid}")
                nc.vector.bn_aggr(mv[:], stats[:])
                sd = lnp.tile([P, 1], FP32, tag="sd", name=f"sd{uid}")
                nc.scalar.activation(sd[:], mv[:, 1:2], AF.Sqrt, bias=epst[:])
                rs = lnp.tile([P, 1], FP32, tag="rs", name=f"rs{uid}")
                nc.vector.reciprocal(rs[:], sd[:])
                nmr = lnp.tile([P, 1], FP32, tag="nmr", name=f"nmr{uid}")
                nc.vector.scalar_tensor_tensor(
                    out=nmr[:],
                    in0=mv[:, 0:1],
                    scalar=-1.0,
                    in1=rs[:],
                    op0=ALU.mult,
                    op1=ALU.mult,
                )
                nc.scalar.activation(
                    out_bf[:], xt[:], AF.Identity, bias=nmr[:], scale=rs[:]
                )

            nT = [g1.tile([P, T], BF16, tag="g1", name=f"nT{i}") for i in range(NCT)]
            nqT = [g3.tile([P, own], BF16, tag="g3", name=f"nqT{i}") for i in range(NCT)]
            kT = [g2.tile([P, T], BF16, tag="g2", name=f"kT{i}") for i in range(NCT)]
            qT = [qwp.tile([P, own], BF16, tag="qw", name=f"qT{i}") for i in range(NCT)]
            vA = []
            for tt in range(NKV):
                v = vap.tile([P, H * 65], BF16, tag="va", name=f"vA{tt}")
                v3 = v.rearrange("p (h k) -> p h k", k=65)
                nc.vector.memset(v3[:, :, 64:65], 1.0)
                vA.append(v)

            _ln_nb = {}

            def ln_load(kt, src, pfx):
                """DMA + LN (Vector/Scalar only, no PE) -> staged nb tile."""
                xt = xin.tile([P, C], FP32, tag="xt", name=f"x{pfx}{kt}")
                nc.sync.dma_start(xt[:], src[P * kt : P * (kt + 1), :])
                nb = nbp.tile([P, C], BF16, tag="nb", name=f"nb{pfx}{kt}")
                layer_norm_to_bf16(xt, nb, f"{pfx}{kt}")
                _ln_nb[(pfx, kt)] = nb

            def ln_trans(kt, dstT, pfx, in_attn=False):
                """8 transposes of a staged nb tile into feature-major dstT."""
                nb = _ln_nb.pop((pfx, kt))
                for ct in range(NCT):
                    if in_attn:
                        pst = ps_small(f"tr{pfx}{kt}_{ct}", (P, P), BF16, True)
                    else:
                        pst = ps.tile(
                            [P, P], BF16, tag="sc", bufs=2, name=f"tr{pfx}{kt}_{ct}"
                        )
                    nc.tensor.transpose(pst[:], nb[:, P * ct : P * (ct + 1)], ident[:])
                    trans_copy(dstT[ct][:, P * kt : P * (kt + 1)], pst[:])

            def keepalive(n, uid):
                """Dummy PE transposes — keep the HAM clock up through
                PE-sparse stretches (results unread)."""
                for i in range(n):
                    wps = ps.tile([P, P], BF16, tag="sc", bufs=2, name=f"ka{uid}_{i}")
                    nc.tensor.transpose(wps[:], ident[:], ident[:])

            def emit_ln1(kt, in_attn=False):
                ln_load(kt, xc, "a")
                ln_trans(kt, nT, "a", in_attn)

            wkb_cache = {}

            def load_wk(ot):
                blks = []
                for ci in range(NCT):
                    w = wsp.tile([P, P], BF16, tag="qk", name=f"wk{ot}_{ci}")
                    nc.sync.dma_start(
                        w[:], wk[P * ci : P * (ci + 1), P * ot : P * (ot + 1)]
                    )
                    blks.append(w)
                wkb_cache.clear()
                wkb_cache[ot] = blks

            def emit_kT(ot, tm, in_attn=False):
                if ot not in wkb_cache:
                    load_wk(ot)
                wkb = wkb_cache[ot]
                p = ps_small(f"kps{ot}_{tm}", in_attn=in_attn)
                for ci in range(NCT):
                    nc.tensor.matmul(
                        p[:],
                        wkb[ci][:],
                        nT[ci][:, 512 * tm : 512 * (tm + 1)],
                        start=(ci == 0),
                        stop=(ci == NCT - 1),
                    )
                if in_attn:
                    nc.vector.tensor_copy(kT[ot][:, 512 * tm : 512 * (tm + 1)], p[:])
                else:
                    nc.scalar.activation(
                        kT[ot][:, 512 * tm : 512 * (tm + 1)], p[:], AF.Copy
                    )

            wvhs = {}

            def load_wv(oj):
                wvh = []
                for ci in range(NCT):
                    w = wvp.tile([P, 512], BF16, tag="v", name=f"wv{oj}_{ci}")
                    nc.sync.dma_start(
                        w[:], wv[P * ci : P * (ci + 1), 512 * oj : 512 * (oj + 1)]
                    )
                    wvh.append(w)
                wvhs[oj] = wvh

            def emit_v(oj, tt, in_attn):
                p = ps_small(f"vps{oj}_{tt}", in_attn=in_attn)
                for ci in range(NCT):
                    nc.tensor.matmul(
                        p[:],
                        nT[ci][:, P * tt : P * (tt + 1)],
                        wvhs[oj][ci][:],
                        start=(ci == 0),
                        stop=(ci == NCT - 1),
                    )
                v3 = vA[tt].rearrange("p (h k) -> p h k", k=65)
                ps3 = p.rearrange("p (h k) -> p h k", k=64)
                if in_attn:
                    nc.vector.tensor_copy(v3[:, 8 * oj : 8 * (oj + 1), 0:64], ps3[:])
                else:
                    nc.scalar.activation(
                        v3[:, 8 * oj : 8 * (oj + 1), 0:64], ps3[:], AF.Copy
                    )

            wqb_cache = {}

            def load_wq(ot, m):
                wqb = []
                for ci in range(NCT):
                    w = wsp.tile([P, P], BF16, tag="qk", name=f"wq{ot}_{m}_{ci}")
                    nc.sync.dma_start(
                        w[:], wq[P * ci : P * (ci + 1), P * ot : P * (ot + 1)]
                    )
                    wqb.append(w)
                wqb_cache.clear()
                wqb_cache[ot] = wqb

            def emit_qT(ot, m, in_attn):
                if ot not in wqb_cache:
                    load_wq(ot, m)
                wqb = wqb_cache[ot]
                p = ps_small(f"qps{ot}_{m}", (P, QM), in_attn=in_attn)
                for ci in range(NCT):
                    nc.tensor.matmul(
                        p[:],
                        wqb[ci][:],
                        nqT[ci][:, QM * m : QM * (m + 1)],
                        start=(ci == 0),
                        stop=(ci == NCT - 1),
                    )
                if in_attn:
                    nc.vector.tensor_copy(qT[ot][:, QM * m : QM * (m + 1)], p[:])
                else:
                    nc.scalar.activation(
                        qT[ot][:, QM * m : QM * (m + 1)], p[:], AF.Copy
                    )

            _sc = nc.enter_named_scope("ph_front", False)[0]
            # ---- front (minimal for m=0 start): LN1 tiles 0-7, A2 0-3,
            # kT[*] first-half context, vA0 0-7, qT m=0. The rest streams in
            # as m=0 fillers.
            load_wv(0)
            for kt in range(4):
                ln_load(kt, xc, "a")
                keepalive(20, f"f{kt}")
                ln_trans(kt, nT, "a")
            emit_kT(0, 0)
            emit_v(0, 0, False)
            emit_v(0, 1, False)
            for kt in range(4, 8):
                ln_load(kt, xc, "a")
                keepalive(12, f"f{kt}")
                ln_trans(kt, nT, "a")
            emit_kT(0, 1)
            for tt in range(2, 6):
                emit_v(0, tt, False)
            emit_kT(1, 0)
            emit_kT(1, 1)
            emit_v(0, 6, False)
            emit_v(0, 7, False)
            for qt in range(0, 2):
                ln_load(qt, xq, "q")
                ln_trans(qt, nqT, "q")
            for ot in (2, 3):
                emit_kT(ot, 0)
                emit_kT(ot, 1)
            for qt in range(2, 4):
                ln_load(qt, xq, "q")
                ln_trans(qt, nqT, "q")
            for ot in (4, 5, 6, 7):
                emit_kT(ot, 0)
                emit_kT(ot, 1)
            for ot in range(NCT):
                emit_qT(ot, 0, False)

            nc.leave_named_scope("ph_front", _sc, False)

            # ---- attention: m outer, hp inner ------------------------------
            attnT = [
                atp.tile([P, own], BF16, tag="at", name=f"attnT{i}") for i in range(NCT)
            ]
            x2 = [None] * NQT
            wc_blks = {}

            def load_wc(oj, phase):
                blks = []
                for ci in range(NCT):
                    w = wcp.tile([P, 512], BF16, tag="wc", name=f"wc{phase}_{oj}_{ci}")
                    nc.sync.dma_start(
                        w[:], wc[P * ci : P * (ci + 1), 512 * oj : 512 * (oj + 1)]
                    )
                    blks.append(w)
                wc_blks[oj] = blks

            def emit_cproj(qt, oj, in_attn=False):
                if x2[qt] is None:
                    x2[qt] = g3.tile([P, C], FP32, tag="g3", name=f"x2_{qt}")
                xr = exq.tile([P, 512], FP32, tag="xr", bufs=1, name=f"xr{qt}_{oj}")
                nc.sync.dma_start(
                    xr[:], xq[P * qt : P * (qt + 1), 512 * oj : 512 * (oj + 1)]
                )
                p = ps_small(f"cps{qt}_{oj}", in_attn=in_attn)
                for ci in range(NCT):
                    nc.tensor.matmul(
                        p[:],
                        attnT[ci][:, P * qt : P * (qt + 1)],
                        wc_blks[oj][ci][:],
                        start=(ci == 0),
                        stop=(ci == NCT - 1),
                    )
                nc.vector.tensor_add(
                    x2[qt][:, 512 * oj : 512 * (oj + 1)], p[:], xr[:]
                )

            # filler queues, popped INSIDE the j-loop (~2us granularity) so PE
            # duty stays high through scalar-bound attention blocks and the
            # HAM clock never drops. Ordering respects data deps.
            def T(f, *a):
                return lambda: f(*a)

            # NOTE ordering: ALL vA oj=0 tiles must be emitted before
            # load_wv(1) — the wv oj=1 DMAs reuse the wvp ring slots whose
            # release requires every vA0 matmul, and the sync engine is
            # in-order (a late vA0 dep would deadlock the DMA queue).
            # q_m0a must be fully emitted before the hp=4 block of m=0 (its
            # AV matmuls read vA oj=1); q_m0b just needs to land within m=0.
            q_m0a = [
                T(ln_load, 8, xc, "a"), T(ln_load, 9, xc, "a"),
                T(ln_trans, 8, nT, "a", True), T(ln_trans, 9, nT, "a", True),
                T(emit_v, 0, 8, True), T(ln_load, 10, xc, "a"),
                T(emit_v, 0, 9, True), T(ln_load, 11, xc, "a"),
                T(ln_trans, 10, nT, "a", True), T(ln_trans, 11, nT, "a", True),
                T(emit_v, 0, 10, True), T(ln_load, 12, xc, "a"),
                T(emit_v, 0, 11, True), T(ln_load, 13, xc, "a"),
                T(ln_trans, 12, nT, "a", True), T(ln_trans, 13, nT, "a", True),
                T(emit_v, 0, 12, True), T(ln_load, 14, xc, "a"),
                T(emit_v, 0, 13, True), T(ln_load, 15, xc, "a"),
                T(ln_trans, 14, nT, "a", True), T(ln_trans, 15, nT, "a", True),
                T(emit_v, 0, 14, True), T(emit_v, 0, 15, True),
                T(load_wv, 1),
            ]
            for tt in range(NKV):
                q_m0a.append(T(emit_v, 1, tt, True))
            q_m0b = []
            for ot in range(NCT):
                q_m0b.append(T(load_wk, ot))
                q_m0b.append(T(emit_kT, ot, 2, True))
                q_m0b.append(T(emit_kT, ot, 3, True))
            for qt in (4, 5):
                q_m0b.append(T(ln_load, qt, xq, "q"))
            for qt in (4, 5):
                q_m0b.append(T(ln_trans, qt, nqT, "q", True))
            for qt in (6, 7):
                q_m0b.append(T(ln_load, qt, xq, "q"))
            for qt in (6, 7):
                q_m0b.append(T(ln_trans, qt, nqT, "q", True))
            for ot in range(NCT):
                q_m0b.append(T(load_wq, ot, 1))
                q_m0b.append(T(emit_qT, ot, 1, True))
            q_m0 = q_m0a + q_m0b
            q_m0a_set = set(q_m0a)

            q_m1 = [T(load_wc, 0, "m1")]
            for qt in range(4):
                q_m1.append(T(emit_cproj, qt, 0, True))
            q_m1.append(T(load_wc, 1, "m1"))
            for qt in range(4):
                q_m1.append(T(emit_cproj, qt, 1, True))


            mask3 = maskt.rearrange("p (r k) -> p r k", r=2)

            def emit_normalize(hp, r, m, avp_r):
                h = 2 * hp + r
                av_s = smp.tile([65, QM], BF16, tag="avs", bufs=2, name=f"avs{h}_{m}")
                with nc.allow_low_precision(reason="attn out + denom to bf16"):
                    nc.vector.tensor_copy(av_s[:], avp_r[:])
                bcp = ps.tile([64, QM], FP32, tag="bc", bufs=1, name=f"bc{h}_{m}")
                nc.tensor.matmul(
                    bcp[:], ones65[64:65, :], av_s[64:65, :], start=True, stop=True
                )
                bcs = smp.tile([64, QM], FP32, tag="bcs", bufs=2, name=f"bcs{h}_{m}")
                nc.vector.reciprocal_approx_fast(bcs[:], bcp[:])
                nc.vector.tensor_mul(
                    attnT[hp][64 * r : 64 * r + 64, QM * m : QM * (m + 1)],
                    av_s[0:64, :],
                    bcs[:],
                )

            _sc = nc.enter_named_scope("ph_attn", False)[0]
            for m in range(NM):
                jmax0 = 2 * QM * (m + 1) // P
                fq = q_m0 if m == 0 else q_m1
                steps_left = [8 * jmax0]

                def pump():
                    """Pop filler thunks, pacing the queue across the m-block."""
                    if not fq:
                        return
                    n = max(1, -(-len(fq) // max(1, steps_left[0])))
                    for _ in range(min(n, 2)):
                        if fq:
                            fq.pop(0)()
                    steps_left[0] -= 1

                for hp in range(H // 2):
                    if m == 0 and hp == 4:
                        # hp>=4 AV matmuls read vA oj=1 — force q_m0a flushed
                        while fq and fq[0] in q_m0a_set:
                            fq.pop(0)()
                    avp = [
                        ps.tile([65, QM], FP32, tag="av", bufs=2, name=f"av{hp}_{m}_{r}")
                        for r in range(2)
                    ]
                    exs = {}

                    def emit_av(j, r):
                        ex, w0 = exs[(j, r)]
                        nc.tensor.matmul(
                            avp[r][:, w0:QM],
                            vA[j][:, 65 * (2 * hp + r) : 65 * (2 * hp + r) + 65],
                            ex[:, QM * r + w0 : QM * (r + 1)],
                            start=(j == 0),
                            stop=(j == jmax0 - 1),
                        )
                        if r == 1:
                            del exs[(j, 0)], exs[(j, 1)]

                    for j in range(jmax0):
                        wq_ = max(0, (P * j - 2 * QM * m) // 2)
                        sc = ps.tile(
                            [P, 2 * QM], FP32, tag="sc", bufs=2,
                            name=f"sc{hp}_{m}_{j}",
                        )
                        for r in range(2):
                            nc.tensor.matmul(
                                sc[:, QM * r + wq_ : QM * (r + 1)],
                                kT[hp][64 * r : 64 * r + 64, P * j : P * (j + 1)],
                                qT[hp][
                                    64 * r : 64 * r + 64, QM * m + wq_ : QM * (m + 1)
                                ],
                                start=True,
                                stop=True,
                            )
                        ex = exq.tile(
                            [P, 2 * QM], BF16, tag="ex", bufs=3,
                            name=f"ex{hp}_{m}_{j}",
                        )
                        sc3 = sc.rearrange("p (r q) -> p r q", r=2)
                        ex3 = ex.rearrange("p (r q) -> p r q", r=2)
                        nc.scalar.activation(
                            ex3[:, :, wq_:QM], sc3[:, :, wq_:QM], AF.Exp
                        )
                        if P * j >= 2 * QM * m:
                            nc.gpsimd.tensor_mul(
                                ex3[:, :, wq_ : wq_ + 64],
                                ex3[:, :, wq_ : wq_ + 64],
                                mask3[:],
                            )
                        exs[(j, 0)] = (ex, wq_)
                        exs[(j, 1)] = (ex, wq_)
                        if j >= 1:
                            emit_av(j - 1, 0)
                            emit_av(j - 1, 1)
                        if m == 0 or j % 8 == 3:
                            pump()
                        if 2 <= j < jmax0:
                            # HAM keepalive: accumulate an all-zero row into
                            # the live softmax psum — dependency-free PE work
                            # so no 3.4us activity window ever reads "idle"
                            # (idle windows halve the PE clock for >=3.4us).
                            zn = 512 if m == 0 else 256
                            for r in range(2):
                                nc.tensor.matmul(
                                    avp[r][:, 0:zn],
                                    zrow[:],
                                    qT[hp][0:1, QM * m : QM * m + zn],
                                    start=False,
                                    stop=False,
                                    skip_group_check=True,
                                )
                    emit_av(jmax0 - 1, 0)
                    emit_av(jmax0 - 1, 1)
                    emit_normalize(hp, 0, m, avp[0])
                    emit_normalize(hp, 1, m, avp[1])
                # everything queued for this m must land before the next m
                fi = 0
                while fq:
                    fq.pop(0)()
                    keepalive(2, f"fl{m}_{fi}")
                    fi += 1
            nc.leave_named_scope("ph_attn", _sc, False)

            _sc = nc.enter_named_scope("ph_post", False)[0]
            # ---- post: c_proj qt4-7 + LN2 interleaved, MLP in f-quarters ---
            mT = [vap.tile([P, own], BF16, tag="va", name=f"mT{i}") for i in range(NCT)]

            def emit_ln2(qt):
                mb = nbp.tile([P, C], BF16, tag="nb", name=f"mb{qt}")
                layer_norm_to_bf16(x2[qt], mb, f"m{qt}")
                for ct in range(NCT):
                    pst = ps_small(f"mtr{qt}_{ct}", (P, P), BF16)
                    nc.tensor.transpose(pst[:], mb[:, P * ct : P * (ct + 1)], ident[:])
                    trans_copy(mT[ct][:, P * qt : P * (qt + 1)], pst[:])

            def load_wfq(fqi):
                wf4 = []
                for k in range(4):
                    w = g2.tile([P, 2048], BF16, tag="g2", name=f"wf{fqi}_{k}")
                    for half in range(2):
                        ci = 2 * k + half
                        nc.sync.dma_start(
                            w[:, 1024 * half : 1024 * (half + 1)],
                            wf[P * ci : P * (ci + 1), 1024 * fqi : 1024 * (fqi + 1)],
                        )
                    wf4.append(w)
                return wf4

            def load_wpq(fqi):
                wp4 = []
                for k in range(4):
                    w = g2.tile([P, 2048], BF16, tag="g2", name=f"wp{fqi}_{k}")
                    for half in range(2):
                        ftl = 2 * k + half
                        r0 = 1024 * fqi + P * ftl
                        nc.sync.dma_start(
                            w[:, 1024 * half : 1024 * (half + 1)], wp[r0 : r0 + P, :]
                        )
                    wp4.append(w)
                return wp4

            def emit_fc(fqi, wf4, hq, ftl, mq):
                p = ps.tile(
                    [P, QM], FP32, tag="sc", bufs=2, name=f"fps{fqi}_{ftl}_{mq}"
                )
                for ci in range(NCT):
                    nc.tensor.matmul(
                        p[:],
                        wf4[ci // 2][
                            :, 1024 * (ci % 2) + P * ftl : 1024 * (ci % 2)
                            + P * (ftl + 1)
                        ],
                        mT[ci][:, QM * mq : QM * (mq + 1)],
                        start=(ci == 0),
                        stop=(ci == NCT - 1),
                    )
                nc.scalar.activation(
                    hq[ftl][:, QM * mq : QM * (mq + 1)], p[:], AF.Gelu_apprx_tanh
                )

            def emit_proj(fqi, wp4, hq, qt):
                for oj in range(2):
                    p = ps_small(f"pps{fqi}_{qt}_{oj}")
                    for ftl in range(8):
                        nc.tensor.matmul(
                            p[:],
                            hq[ftl][:, P * qt : P * (qt + 1)],
                            wp4[ftl // 2][
                                :, 1024 * (ftl % 2) + 512 * oj : 1024 * (ftl % 2)
                                + 512 * (oj + 1)
                            ],
                            start=(ftl == 0),
                            stop=(ftl == 7),
                        )
                    nc.vector.tensor_add(
                        x2[qt][:, 512 * oj : 512 * (oj + 1)],
                        p[:],
                        x2[qt][:, 512 * oj : 512 * (oj + 1)],
                    )

            # start of post: c_proj(qt4-7) rides over LN2's vector work; the
            # first fc chains overlap LN2 of qt4-7.
            load_wc(0, "p")
            for qt in range(4, 8):
                emit_cproj(qt, 0)
                keepalive(3, f"pc{qt}")
                emit_ln2(qt - 4)
                keepalive(3, f"pl{qt}")
            load_wc(1, "p")
            wf4_0 = load_wfq(0)
            wp4_0 = load_wpq(0)
            for qt in range(4, 8):
                emit_cproj(qt, 1)
                keepalive(3, f"pd{qt}")
            hq0 = [g1.tile([P, own], BF16, tag="g1", name=f"hq0_{i}") for i in range(8)]
            for ftl in range(8):
                emit_fc(0, wf4_0, hq0, ftl, 0)
                if ftl < 4:
                    emit_ln2(4 + ftl)
            for ftl in range(8):
                emit_fc(0, wf4_0, hq0, ftl, 1)
            for qt in range(NQT):
                emit_proj(0, wp4_0, hq0, qt)

            for fqi in range(1, 4):
                wf4 = load_wfq(fqi)
                wp4 = load_wpq(fqi)
                hq = [
                    g1.tile([P, own], BF16, tag="g1", name=f"hq{fqi}_{i}")
                    for i in range(8)
                ]
                for ftl in range(8):
                    for mq in range(2):
                        emit_fc(fqi, wf4, hq, ftl, mq)
                for qt in range(NQT):
                    emit_proj(fqi, wp4, hq, qt)
            for qt in range(NQT):
                nc.sync.dma_start(yout[P * qt : P * (qt + 1), :], x2[qt][:])
            nc.leave_named_scope("ph_post", _sc, False)

    nc.compile()
    return nc


def stage_inputs(x, c_attn_w, c_proj_w, fc_w, proj_w, ln1_g, ln2_g, T=2048, n_cores=8):
    """Host-side prep: per-core input maps. x: (B, T, C) f32."""
    bf = ml_dtypes.bfloat16
    g1w = c_attn_w * ln1_g[:, None]
    wqh = np.ascontiguousarray((g1w[:, 0:C] * 0.125).astype(bf))
    wkh = np.ascontiguousarray(g1w[:, C : 2 * C].astype(bf))
    wvh = np.ascontiguousarray(g1w[:, 2 * C : 3 * C].astype(bf))
    wch = np.ascontiguousarray(c_proj_w.astype(bf))
    wfh = np.ascontiguousarray((fc_w * ln2_g[:, None]).astype(bf))
    wph = np.ascontiguousarray(proj_w.astype(bf))
    in_maps = []
    for c in range(n_cores):
        b, s = c // 2, c % 2
        xcv = np.ascontiguousarray(x[b][:T], dtype=np.float32)
        xqv = np.ascontiguousarray(x[b][s:T:2], dtype=np.float32)
        kvl = np.arange(P)[:, None]
        ul = np.arange(64)[None, :]
        mask = (2 * ul + s >= kvl).astype(np.float32)
        mask = np.tile(mask, (1, 2))
        in_maps.append(
            {
                "xc": xcv,
                "xq": xqv,
                "wq": wqh,
                "wk": wkh,
                "wv": wvh,
                "wc": wch,
                "wf": wfh,
                "wp": wph,
                "msk": mask.astype(bf),
            }
        )
    return in_maps


_NC_CACHE = {}


def _get_nc(T=2048):
    if T not in _NC_CACHE:
        _NC_CACHE[T] = build_nc(T=T)
    return _NC_CACHE[T]


def kernel(**inputs):
    """Full transformer block on 8 NeuronCores. Takes/returns full numpy arrays."""
    from concourse.bass_utils import run_bass_kernel_spmd

    x = np.asarray(inputs["x"], dtype=np.float32)
    B, T, C_ = x.shape
    nc = _get_nc(T=T)
    in_maps = stage_inputs(
        x,
        np.asarray(inputs["c_attn_w"], dtype=np.float32),
        np.asarray(inputs["c_proj_w"], dtype=np.float32),
        np.asarray(inputs["fc_w"], dtype=np.float32),
        np.asarray(inputs["proj_w"], dtype=np.float32),
        np.asarray(inputs["ln1_g"], dtype=np.float32),
        np.asarray(inputs["ln2_g"], dtype=np.float32),
        T=T,
        n_cores=8,
    )
    res = run_bass_kernel_spmd(nc, in_maps, list(range(8)))
    out = np.empty((B, T, C_), dtype=np.float32)
    for c in range(8):
        b, s = c // 2, c % 2
        out[b, s::2, :] = res.results[c]["yout"]
    return out


# revision 44
# speedup vs baseline: 1.1237x; 1.1237x over previous
"""Transformer block (B=4,T=2048,C=1024,H=16) on 8 trn2 cores, zero-communication.

Split: core c -> sequence b=c//2, token parity s=c%2. Each core computes the
full block output for its 1024 strided query tokens (positions s, s+2, ...),
recomputing LN1+K/V for the whole 2048-token context locally (no collectives).

Pipeline (v2): phases are software-pipelined to keep TensorE (and the HAM
clock) saturated end to end:
  front: LN1 tiles interleaved with kT / vA(oj=0) / qT(m=0) projections
  m=0:   attention q-cols 0..511 for all head-pairs; fillers = vA(oj=1),
         qT(m=1)
  m=1:   attention q-cols 512..1023; fillers = c_proj + residual for token
         tiles 0..3
  post:  c_proj qt 4..7, LN2 -> mT, then MLP in 4 f-quarters with
         double-buffered wf/wp streaming; proj accumulates into x2 in SBUF.

PSUM (8 banks): sc 2x[128,1024] (4) | av 2x[*,512] (2) | bc 1 (1) | fill 1 (1).
Softmax normalize: avp -> SBUF early-release copy, denom row broadcast by
matmul, reciprocal_approx_fast, tensor_mul.
"""
import sys

sys.path.insert(0, "/opt/trn_rl_repo")

import numpy as np
import ml_dtypes

import concourse.bass as bass
import concourse.mybir as mybir
import concourse.tile as tile
from concourse import bacc
from concourse.masks import make_identity

FP32 = mybir.dt.float32
BF16 = mybir.dt.bfloat16
AF = mybir.ActivationFunctionType
ALU = mybir.AluOpType

C = 1024
H = 16
HS = 64
FF = 4096
LN_EPS = 1e-5
P = 128


def build_nc(T=2048):
    own = T // 2          # query tokens per core
    NKV = T // P          # kv token tiles (16)
    QM = min(512, own)    # q-macro width
    NM = own // QM        # q macros (2)
    NQT = own // P        # own-token tiles (8)
    NCT = C // P          # feature tiles (8)
    NTM = T // 512        # kv 512-macros (4)

    nc = bacc.Bacc(None, target_bir_lowering=False, debug=False)

    xc = nc.dram_tensor("xc", [T, C], FP32, kind="ExternalInput")
    xq = nc.dram_tensor("xq", [own, C], FP32, kind="ExternalInput")
    wq = nc.dram_tensor("wq", [C, C], BF16, kind="ExternalInput")
    wk = nc.dram_tensor("wk", [C, C], BF16, kind="ExternalInput")
    wv = nc.dram_tensor("wv", [C, C], BF16, kind="ExternalInput")
    wc = nc.dram_tensor("wc", [C, C], BF16, kind="ExternalInput")
    wf = nc.dram_tensor("wf", [C, FF], BF16, kind="ExternalInput")
    wp = nc.dram_tensor("wp", [FF, C], BF16, kind="ExternalInput")
    msk = nc.dram_tensor("msk", [P, 2 * 64], BF16, kind="ExternalInput")
    yout = nc.dram_tensor("yout", [own, C], FP32, kind="ExternalOutput")

    with tile.TileContext(nc) as tc:
        import contextlib

        with contextlib.ExitStack() as ctx:
            const = ctx.enter_context(tc.tile_pool(name="const", bufs=1))
            xin = ctx.enter_context(tc.tile_pool(name="xin", bufs=2))
            lnp = ctx.enter_context(tc.tile_pool(name="lnp", bufs=3))
            nbp = ctx.enter_context(tc.tile_pool(name="nbp", bufs=2))
            g1 = ctx.enter_context(tc.tile_pool(name="g1", bufs=NCT))   # nT -> hq
            g2 = ctx.enter_context(tc.tile_pool(name="g2", bufs=NCT))   # kT -> wf/wp
            g3 = ctx.enter_context(tc.tile_pool(name="g3", bufs=NCT))   # nqT -> x2
            vap = ctx.enter_context(tc.tile_pool(name="vap", bufs=NKV))  # vA -> mT
            qwp = ctx.enter_context(tc.tile_pool(name="qwp", bufs=NCT))  # qT
            wcp = ctx.enter_context(tc.tile_pool(name="wcp", bufs=NCT))  # wc halves
            atp = ctx.enter_context(tc.tile_pool(name="atp", bufs=NCT))  # attnT
            exq = ctx.enter_context(tc.tile_pool(name="exq", bufs=3))    # ex + xr
            smp = ctx.enter_context(tc.tile_pool(name="smp", bufs=3))    # softmax
            wsp = ctx.enter_context(tc.tile_pool(name="wsp", bufs=8))    # wq/wk blk
            wvp = ctx.enter_context(tc.tile_pool(name="wvp", bufs=8))   # wv halves

            ps = ctx.enter_context(tc.tile_pool(name="ps", bufs=1, space="PSUM"))

            _psn = [0, 0]

            def ps_small(name, shape=(P, 512), dtype=FP32, in_attn=False):
                """Rotating one-bank psum tile.

                Outside attention: cycles av/bc/fill (ring of 4 incl. av's 2
                slots). Inside attention the av slots are HELD by the live
                softmax accumulators — allocating av there deadlocks the PE —
                so fillers cycle bc/fill only."""
                if in_attn:
                    tag = ("bc", "fill")[_psn[1] % 2]
                    _psn[1] += 1
                else:
                    tag = ("av", "bc", "fill")[_psn[0] % 3]
                    _psn[0] += 1
                bufs = {"av": 2, "bc": 1, "fill": 1}[tag]
                return ps.tile(list(shape), dtype, tag=tag, bufs=bufs, name=name)

            ident = const.tile([P, P], BF16)
            make_identity(nc, ident)
            ones65 = const.tile([65, 64], BF16)
            nc.vector.memset(ones65[64:65, :], 1.0)
            zrow = const.tile([1, 65], BF16)
            nc.vector.memset(zrow, 0.0)
            epst = const.tile([P, 1], FP32)
            nc.vector.memset(epst, LN_EPS)
            maskt = const.tile([P, 2 * 64], BF16)
            nc.sync.dma_start(maskt[:], msk[:])

            # PE warmup: HAM clock ramp while the first LN chunks stream in.
            for wi in range(48):
                wps = ps.tile([P, P], BF16, tag="sc", bufs=2, name=f"warm{wi}")
                nc.tensor.transpose(wps[:], ident[:], ident[:])

            _ceng = [0]

            def trans_copy(dst, src):
                """PSUM->SBUF [128,128] copy, alternated scalar/vector
                (GpSimd cannot read PSUM)."""
                k = _ceng[0] % 2
                _ceng[0] += 1
                if k == 0:
                    nc.scalar.activation(dst, src, AF.Copy)
                else:
                    nc.vector.tensor_copy(dst, src)

            def layer_norm_to_bf16(xt, out_bf, uid):
                """xt [128, C] f32 -> out_bf [128, C] bf16 (normalized).

                Stats on VectorE; the big apply on ScalarE via
                Identity(x*rs + (-mu*rs))."""
                stats = lnp.tile([P, 2, 6], FP32, tag="stats", name=f"st{uid}")
                nc.vector.bn_stats(stats[:, 0, :], xt[:, 0:512])
                nc.vector.bn_stats(stats[:, 1, :], xt[:, 512:1024])
                mv = lnp.tile([P, 2], FP32, tag="mv", name=f"mv{uid}")
                nc.vector.bn_aggr(mv[:], stats[:])
                sd = lnp.tile([P, 1], FP32, tag="sd", name=f"sd{uid}")
                nc.scalar.activation(sd[:], mv[:, 1:2], AF.Sqrt, bias=epst[:])
                rs = lnp.tile([P, 1], FP32, tag="rs", name=f"rs{uid}")
                nc.vector.reciprocal(rs[:], sd[:])
                nmr = lnp.tile([P, 1], FP32, tag="nmr", name=f"nmr{uid}")
                nc.vector.scalar_tensor_tensor(
                    out=nmr[:],
                    in0=mv[:, 0:1],
                    scalar=-1.0,
                    in1=rs[:],
                    op0=ALU.mult,
                    op1=ALU.mult,
                )
                nc.scalar.activation(
                    out_bf[:], xt[:], AF.Identity, bias=nmr[:], scale=rs[:]
                )

            nT = [g1.tile([P, T], BF16, tag="g1", name=f"nT{i}") for i in range(NCT)]
            nqT = [g3.tile([P, own], BF16, tag="g3", name=f"nqT{i}") for i in range(NCT)]
            kT = [g2.tile([P, T], BF16, tag="g2", name=f"kT{i}") for i in range(NCT)]
            qT = [qwp.tile([P, own], BF16, tag="qw", name=f"qT{i}") for i in range(NCT)]
            vA = []
            for tt in range(NKV):
                v = vap.tile([P, H * 65], BF16, tag="va", name=f"vA{tt}")
                v3 = v.rearrange("p (h k) -> p h k", k=65)
                nc.vector.memset(v3[:, :, 64:65], 1.0)
                vA.append(v)

            _ln_nb = {}

            def ln_load(kt, src, pfx):
                """DMA + LN (Vector/Scalar only, no PE) -> staged nb tile."""
                xt = xin.tile([P, C], FP32, tag="xt", name=f"x{pfx}{kt}")
                nc.sync.dma_start(xt[:], src[P * kt : P * (kt + 1), :])
                nb = nbp.tile([P, C], BF16, tag="nb", name=f"nb{pfx}{kt}")
                layer_norm_to_bf16(xt, nb, f"{pfx}{kt}")
                _ln_nb[(pfx, kt)] = nb

            def ln_trans(kt, dstT, pfx, in_attn=False):
                """8 transposes of a staged nb tile into feature-major dstT."""
                nb = _ln_nb.pop((pfx, kt))
                for ct in range(NCT):
                    if in_attn:
                        pst = ps_small(f"tr{pfx}{kt}_{ct}", (P, P), BF16, True)
                    else:
                        pst = ps.tile(
                            [P, P], BF16, tag="sc", bufs=2, name=f"tr{pfx}{kt}_{ct}"
                        )
                    nc.tensor.transpose(pst[:], nb[:, P * ct : P * (ct + 1)], ident[:])
                    trans_copy(dstT[ct][:, P * kt : P * (kt + 1)], pst[:])

            def keepalive(n, uid):
                """Dummy PE transposes — keep the HAM clock up through
                PE-sparse stretches (results unread)."""
                for i in range(n):
                    wps = ps.tile([P, P], BF16, tag="sc", bufs=2, name=f"ka{uid}_{i}")
                    nc.tensor.transpose(wps[:], ident[:], ident[:])

            def emit_ln1(kt, in_attn=False):
                ln_load(kt, xc, "a")
                ln_trans(kt, nT, "a", in_attn)

            wkb_cache = {}

            def load_wk(ot):
                blks = []
                for ci in range(NCT):
                    w = wsp.tile([P, P], BF16, tag="qk", name=f"wk{ot}_{ci}")
                    nc.sync.dma_start(
                        w[:], wk[P * ci : P * (ci + 1), P * ot : P * (ot + 1)]
                    )
                    blks.append(w)
                wkb_cache.clear()
                wkb_cache[ot] = blks

            def emit_kT(ot, tm, in_attn=False):
                if ot not in wkb_cache:
                    load_wk(ot)
                wkb = wkb_cache[ot]
                p = ps_small(f"kps{ot}_{tm}", in_attn=in_attn)
                for ci in range(NCT):
                    nc.tensor.matmul(
                        p[:],
                        wkb[ci][:],
                        nT[ci][:, 512 * tm : 512 * (tm + 1)],
                        start=(ci == 0),
                        stop=(ci == NCT - 1),
                    )
                if in_attn:
                    nc.vector.tensor_copy(kT[ot][:, 512 * tm : 512 * (tm + 1)], p[:])
                else:
                    nc.scalar.activation(
                        kT[ot][:, 512 * tm : 512 * (tm + 1)], p[:], AF.Copy
                    )

            wvhs = {}

            def load_wv(oj):
                wvh = []
                for ci in range(NCT):
                    w = wvp.tile([P, 512], BF16, tag="v", name=f"wv{oj}_{ci}")
                    nc.sync.dma_start(
                        w[:], wv[P * ci : P * (ci + 1), 512 * oj : 512 * (oj + 1)]
                    )
                    wvh.append(w)
                wvhs[oj] = wvh

            def emit_v(oj, tt, in_attn):
                p = ps_small(f"vps{oj}_{tt}", in_attn=in_attn)
                for ci in range(NCT):
                    nc.tensor.matmul(
                        p[:],
                        nT[ci][:, P * tt : P * (tt + 1)],
                        wvhs[oj][ci][:],
                        start=(ci == 0),
                        stop=(ci == NCT - 1),
                    )
                v3 = vA[tt].rearrange("p (h k) -> p h k", k=65)
                ps3 = p.rearrange("p (h k) -> p h k", k=64)
                if in_attn:
                    nc.vector.tensor_copy(v3[:, 8 * oj : 8 * (oj + 1), 0:64], ps3[:])
                else:
                    nc.scalar.activation(
                        v3[:, 8 * oj : 8 * (oj + 1), 0:64], ps3[:], AF.Copy
                    )

            wqb_cache = {}

            def load_wq(ot, m):
                wqb = []
                for ci in range(NCT):
                    w = wsp.tile([P, P], BF16, tag="qk", name=f"wq{ot}_{m}_{ci}")
                    nc.sync.dma_start(
                        w[:], wq[P * ci : P * (ci + 1), P * ot : P * (ot + 1)]
                    )
                    wqb.append(w)
                wqb_cache.clear()
                wqb_cache[ot] = wqb

            def emit_qT(ot, m, in_attn):
                if ot not in wqb_cache:
                    load_wq(ot, m)
                wqb = wqb_cache[ot]
                p = ps_small(f"qps{ot}_{m}", (P, QM), in_attn=in_attn)
                for ci in range(NCT):
                    nc.tensor.matmul(
                        p[:],
                        wqb[ci][:],
                        nqT[ci][:, QM * m : QM * (m + 1)],
                        start=(ci == 0),
                        stop=(ci == NCT - 1),
                    )
                if in_attn:
                    nc.vector.tensor_copy(qT[ot][:, QM * m : QM * (m + 1)], p[:])
                else:
                    nc.scalar.activation(
                        qT[ot][:, QM * m : QM * (m + 1)], p[:], AF.Copy
                    )

            _sc = nc.enter_named_scope("ph_front", False)[0]
            # ---- front (minimal for m=0 start): LN1 tiles 0-7, A2 0-3,
            # kT[*] first-half context, vA0 0-7, qT m=0. The rest streams in
            # as m=0 fillers.
            load_wv(0)
            for kt in range(4):
                ln_load(kt, xc, "a")
                keepalive(20, f"f{kt}")
                ln_trans(kt, nT, "a")
            emit_kT(0, 0)
            emit_v(0, 0, False)
            emit_v(0, 1, False)
            for kt in range(4, 8):
                ln_load(kt, xc, "a")
                keepalive(12, f"f{kt}")
                ln_trans(kt, nT, "a")
            emit_kT(0, 1)
            for tt in range(2, 6):
                emit_v(0, tt, False)
            emit_kT(1, 0)
            emit_kT(1, 1)
            emit_v(0, 6, False)
            emit_v(0, 7, False)
            for qt in range(0, 2):
                ln_load(qt, xq, "q")
                ln_trans(qt, nqT, "q")
            for ot in (2, 3):
                emit_kT(ot, 0)
                emit_kT(ot, 1)
            for qt in range(2, 4):
                ln_load(qt, xq, "q")
                ln_trans(qt, nqT, "q")
            for ot in (4, 5, 6, 7):
                emit_kT(ot, 0)
                emit_kT(ot, 1)
            for ot in range(NCT):
                emit_qT(ot, 0, False)

            nc.leave_named_scope("ph_front", _sc, False)

            # ---- attention: m outer, hp inner ------------------------------
            attnT = [
                atp.tile([P, own], BF16, tag="at", name=f"attnT{i}") for i in range(NCT)
            ]
            x2 = [None] * NQT
            wc_blks = {}

            def load_wc(oj, phase):
                blks = []
                for ci in range(NCT):
                    w = wcp.tile([P, 512], BF16, tag="wc", name=f"wc{phase}_{oj}_{ci}")
                    nc.sync.dma_start(
                        w[:], wc[P * ci : P * (ci + 1), 512 * oj : 512 * (oj + 1)]
                    )
                    blks.append(w)
                wc_blks[oj] = blks

            def emit_cproj(qt, oj, in_attn=False):
                if x2[qt] is None:
                    x2[qt] = g3.tile([P, C], FP32, tag="g3", name=f"x2_{qt}")
                xr = exq.tile([P, 512], FP32, tag="xr", bufs=1, name=f"xr{qt}_{oj}")
                nc.sync.dma_start(
                    xr[:], xq[P * qt : P * (qt + 1), 512 * oj : 512 * (oj + 1)]
                )
                p = ps_small(f"cps{qt}_{oj}", in_attn=in_attn)
                for ci in range(NCT):
                    nc.tensor.matmul(
                        p[:],
                        attnT[ci][:, P * qt : P * (qt + 1)],
                        wc_blks[oj][ci][:],
                        start=(ci == 0),
                        stop=(ci == NCT - 1),
                    )
                nc.vector.tensor_add(
                    x2[qt][:, 512 * oj : 512 * (oj + 1)], p[:], xr[:]
                )

            # filler queues, popped INSIDE the j-loop (~2us granularity) so PE
            # duty stays high through scalar-bound attention blocks and the
            # HAM clock never drops. Ordering respects data deps.
            def T(f, *a):
                return lambda: f(*a)

            # NOTE ordering: ALL vA oj=0 tiles must be emitted before
            # load_wv(1) — the wv oj=1 DMAs reuse the wvp ring slots whose
            # release requires every vA0 matmul, and the sync engine is
            # in-order (a late vA0 dep would deadlock the DMA queue).
            # q_m0a must be fully emitted before the hp=4 block of m=0 (its
            # AV matmuls read vA oj=1); q_m0b just needs to land within m=0.
            q_m0a = [
                T(ln_load, 8, xc, "a"), T(ln_load, 9, xc, "a"),
                T(ln_trans, 8, nT, "a", True), T(ln_trans, 9, nT, "a", True),
                T(emit_v, 0, 8, True), T(ln_load, 10, xc, "a"),
                T(emit_v, 0, 9, True), T(ln_load, 11, xc, "a"),
                T(ln_trans, 10, nT, "a", True), T(ln_trans, 11, nT, "a", True),
                T(emit_v, 0, 10, True), T(ln_load, 12, xc, "a"),
                T(emit_v, 0, 11, True), T(ln_load, 13, xc, "a"),
                T(ln_trans, 12, nT, "a", True), T(ln_trans, 13, nT, "a", True),
                T(emit_v, 0, 12, True), T(ln_load, 14, xc, "a"),
                T(emit_v, 0, 13, True), T(ln_load, 15, xc, "a"),
                T(ln_trans, 14, nT, "a", True), T(ln_trans, 15, nT, "a", True),
                T(emit_v, 0, 14, True), T(emit_v, 0, 15, True),
                T(load_wv, 1),
            ]
            for tt in range(NKV):
                q_m0a.append(T(emit_v, 1, tt, True))
            q_m0b = []
            for ot in range(NCT):
                q_m0b.append(T(load_wk, ot))
                q_m0b.append(T(emit_kT, ot, 2, True))
                q_m0b.append(T(emit_kT, ot, 3, True))
            for qt in (4, 5):
                q_m0b.append(T(ln_load, qt, xq, "q"))
            for qt in (4, 5):
                q_m0b.append(T(ln_trans, qt, nqT, "q", True))
            for qt in (6, 7):
                q_m0b.append(T(ln_load, qt, xq, "q"))
            for qt in (6, 7):
                q_m0b.append(T(ln_trans, qt, nqT, "q", True))
            for ot in range(NCT):
                q_m0b.append(T(load_wq, ot, 1))
                q_m0b.append(T(emit_qT, ot, 1, True))
            q_m0 = q_m0a + q_m0b
            q_m0a_set = set(q_m0a)

            q_m1 = [T(load_wc, 0, "m1")]
            for qt in range(4):
                q_m1.append(T(emit_cproj, qt, 0, True))
            q_m1.append(T(load_wc, 1, "m1"))
            for qt in range(4):
                q_m1.append(T(emit_cproj, qt, 1, True))


            mask3 = maskt.rearrange("p (r k) -> p r k", r=2)

            _pending_norms = []

            def emit_normalize(hp, r, m, avp_r):
                """Part A: early-release copy of the accumulator to SBUF.
                The PE-side broadcast + divide (part B) is deferred into the
                NEXT block's pipeline so the block boundary has no
                cross-engine dependency gap on the PE."""
                h = 2 * hp + r
                av_s = smp.tile([65, QM], BF16, tag="avs", bufs=2, name=f"avs{h}_{m}")
                with nc.allow_low_precision(reason="attn out + denom to bf16"):
                    nc.vector.tensor_copy(av_s[:], avp_r[:])
                _pending_norms.append((hp, r, m, av_s))

            def flush_norms():
                while _pending_norms:
                    hp, r, m, av_s = _pending_norms.pop(0)
                    h = 2 * hp + r
                    bcp = ps.tile([64, QM], FP32, tag="bc", bufs=1, name=f"bc{h}_{m}")
                    nc.tensor.matmul(
                        bcp[:], ones65[64:65, :], av_s[64:65, :], start=True, stop=True
                    )
                    bcs = smp.tile(
                        [64, QM], FP32, tag="bcs", bufs=2, name=f"bcs{h}_{m}"
                    )
                    nc.vector.reciprocal_approx_fast(bcs[:], bcp[:])
                    nc.vector.tensor_mul(
                        attnT[hp][64 * r : 64 * r + 64, QM * m : QM * (m + 1)],
                        av_s[0:64, :],
                        bcs[:],
                    )

            _sc = nc.enter_named_scope("ph_attn", False)[0]
            for m in range(NM):
                jmax0 = 2 * QM * (m + 1) // P
                fq = q_m0 if m == 0 else q_m1
                steps_left = [8 * jmax0]

                def pump():
                    """Pop filler thunks, pacing the queue across the m-block."""
                    if not fq:
                        return
                    n = max(1, -(-len(fq) // max(1, steps_left[0])))
                    for _ in range(min(n, 2)):
                        if fq:
                            fq.pop(0)()
                    steps_left[0] -= 1

                for hp in range(H // 2):
                    if m == 0 and hp == 4:
                        # hp>=4 AV matmuls read vA oj=1 — force q_m0a flushed
                        while fq and fq[0] in q_m0a_set:
                            fq.pop(0)()
                    avp = [
                        ps.tile([65, QM], FP32, tag="av", bufs=2, name=f"av{hp}_{m}_{r}")
                        for r in range(2)
                    ]
                    exs = {}

                    def emit_av(j, r):
                        ex, w0 = exs[(j, r)]
                        nc.tensor.matmul(
                            avp[r][:, w0:QM],
                            vA[j][:, 65 * (2 * hp + r) : 65 * (2 * hp + r) + 65],
                            ex[:, QM * r + w0 : QM * (r + 1)],
                            start=(j == 0),
                            stop=(j == jmax0 - 1),
                        )
                        if r == 1:
                            del exs[(j, 0)], exs[(j, 1)]

                    for j in range(jmax0):
                        wq_ = max(0, (P * j - 2 * QM * m) // 2)
                        sc = ps.tile(
                            [P, 2 * QM], FP32, tag="sc", bufs=2,
                            name=f"sc{hp}_{m}_{j}",
                        )
                        for r in range(2):
                            nc.tensor.matmul(
                                sc[:, QM * r + wq_ : QM * (r + 1)],
                                kT[hp][64 * r : 64 * r + 64, P * j : P * (j + 1)],
                                qT[hp][
                                    64 * r : 64 * r + 64, QM * m + wq_ : QM * (m + 1)
                                ],
                                start=True,
                                stop=True,
                            )
                        ex = exq.tile(
                            [P, 2 * QM], BF16, tag="ex", bufs=3,
                            name=f"ex{hp}_{m}_{j}",
                        )
                        sc3 = sc.rearrange("p (r q) -> p r q", r=2)
                        ex3 = ex.rearrange("p (r q) -> p r q", r=2)
                        nc.scalar.activation(
                            ex3[:, :, wq_:QM], sc3[:, :, wq_:QM], AF.Exp
                        )
                        if P * j >= 2 * QM * m:
                            nc.gpsimd.tensor_mul(
                                ex3[:, :, wq_ : wq_ + 64],
                                ex3[:, :, wq_ : wq_ + 64],
                                mask3[:],
                            )
                        exs[(j, 0)] = (ex, wq_)
                        exs[(j, 1)] = (ex, wq_)
                        if j >= 1:
                            emit_av(j - 1, 0)
                            emit_av(j - 1, 1)
                        if m == 0 or j % 8 == 3:
                            pump()
                        if j == 2:
                            flush_norms()
                    emit_av(jmax0 - 1, 0)
                    emit_av(jmax0 - 1, 1)
                    if m == 0:
                        pump()  # dep-free PE filler bridges the boundary
                    emit_normalize(hp, 0, m, avp[0])
                    emit_normalize(hp, 1, m, avp[1])
                # everything queued for this m must land before the next m
                while fq:
                    fq.pop(0)()
            flush_norms()
            nc.leave_named_scope("ph_attn", _sc, False)

            _sc = nc.enter_named_scope("ph_post", False)[0]
            # ---- post: c_proj qt4-7 + LN2 interleaved, MLP in f-quarters ---
            mT = [vap.tile([P, own], BF16, tag="va", name=f"mT{i}") for i in range(NCT)]

            def emit_ln2(qt):
                mb = nbp.tile([P, C], BF16, tag="nb", name=f"mb{qt}")
                layer_norm_to_bf16(x2[qt], mb, f"m{qt}")
                for ct in range(NCT):
                    pst = ps_small(f"mtr{qt}_{ct}", (P, P), BF16)
                    nc.tensor.transpose(pst[:], mb[:, P * ct : P * (ct + 1)], ident[:])
                    trans_copy(mT[ct][:, P * qt : P * (qt + 1)], pst[:])

            def load_wfq(fqi):
                wf4 = []
                for k in range(4):
                    w = g2.tile([P, 2048], BF16, tag="g2", name=f"wf{fqi}_{k}")
                    for half in range(2):
                        ci = 2 * k + half
                        nc.sync.dma_start(
                            w[:, 1024 * half : 1024 * (half + 1)],
                            wf[P * ci : P * (ci + 1), 1024 * fqi : 1024 * (fqi + 1)],
                        )
                    wf4.append(w)
                return wf4

            def load_wpq(fqi):
                wp4 = []
                for k in range(4):
                    w = g2.tile([P, 2048], BF16, tag="g2", name=f"wp{fqi}_{k}")
                    for half in range(2):
                        ftl = 2 * k + half
                        r0 = 1024 * fqi + P * ftl
                        nc.sync.dma_start(
                            w[:, 1024 * half : 1024 * (half + 1)], wp[r0 : r0 + P, :]
                        )
                    wp4.append(w)
                return wp4

            def emit_fc(fqi, wf4, hq, ftl, mq):
                p = ps.tile(
                    [P, QM], FP32, tag="sc", bufs=2, name=f"fps{fqi}_{ftl}_{mq}"
                )
                for ci in range(NCT):
                    nc.tensor.matmul(
                        p[:],
                        wf4[ci // 2][
                            :, 1024 * (ci % 2) + P * ftl : 1024 * (ci % 2)
                            + P * (ftl + 1)
                        ],
                        mT[ci][:, QM * mq : QM * (mq + 1)],
                        start=(ci == 0),
                        stop=(ci == NCT - 1),
                    )
                nc.scalar.activation(
                    hq[ftl][:, QM * mq : QM * (mq + 1)], p[:], AF.Gelu_apprx_tanh
                )

            def emit_proj(fqi, wp4, hq, qt):
                for oj in range(2):
                    p = ps_small(f"pps{fqi}_{qt}_{oj}")
                    for ftl in range(8):
                        nc.tensor.matmul(
                            p[:],
                            hq[ftl][:, P * qt : P * (qt + 1)],
                            wp4[ftl // 2][
                                :, 1024 * (ftl % 2) + 512 * oj : 1024 * (ftl % 2)
                                + 512 * (oj + 1)
                            ],
                            start=(ftl == 0),
                            stop=(ftl == 7),
                        )
                    nc.vector.tensor_add(
                        x2[qt][:, 512 * oj : 512 * (oj + 1)],
                        p[:],
                        x2[qt][:, 512 * oj : 512 * (oj + 1)],
                    )

            # start of post: c_proj(qt4-7) rides over LN2's vector work; the
            # first fc chains overlap LN2 of qt4-7.
            load_wc(0, "p")
            for qt in range(4, 8):
                emit_cproj(qt, 0)
                emit_ln2(qt - 4)
            load_wc(1, "p")
            wf4_0 = load_wfq(0)
            wp4_0 = load_wpq(0)
            for qt in range(4, 8):
                emit_cproj(qt, 1)
            hq0 = [g1.tile([P, own], BF16, tag="g1", name=f"hq0_{i}") for i in range(8)]
            for ftl in range(8):
                emit_fc(0, wf4_0, hq0, ftl, 0)
                if ftl < 4:
                    emit_ln2(4 + ftl)
            for ftl in range(8):
                emit_fc(0, wf4_0, hq0, ftl, 1)
            for qt in range(NQT):
                emit_proj(0, wp4_0, hq0, qt)

            for fqi in range(1, 4):
                wf4 = load_wfq(fqi)
                wp4 = load_wpq(fqi)
                hq = [
                    g1.tile([P, own], BF16, tag="g1", name=f"hq{fqi}_{i}")
                    for i in range(8)
                ]
                for ftl in range(8):
                    for mq in range(2):
                        emit_fc(fqi, wf4, hq, ftl, mq)
                for qt in range(NQT):
                    emit_proj(fqi, wp4, hq, qt)
            for qt in range(NQT):
                nc.sync.dma_start(yout[P * qt : P * (qt + 1), :], x2[qt][:])
            nc.leave_named_scope("ph_post", _sc, False)

    nc.compile()
    return nc


def stage_inputs(x, c_attn_w, c_proj_w, fc_w, proj_w, ln1_g, ln2_g, T=2048, n_cores=8):
    """Host-side prep: per-core input maps. x: (B, T, C) f32."""
    bf = ml_dtypes.bfloat16
    g1w = c_attn_w * ln1_g[:, None]
    wqh = np.ascontiguousarray((g1w[:, 0:C] * 0.125).astype(bf))
    wkh = np.ascontiguousarray(g1w[:, C : 2 * C].astype(bf))
    wvh = np.ascontiguousarray(g1w[:, 2 * C : 3 * C].astype(bf))
    wch = np.ascontiguousarray(c_proj_w.astype(bf))
    wfh = np.ascontiguousarray((fc_w * ln2_g[:, None]).astype(bf))
    wph = np.ascontiguousarray(proj_w.astype(bf))
    in_maps = []
    for c in range(n_cores):
        b, s = c // 2, c % 2
        xcv = np.ascontiguousarray(x[b][:T], dtype=np.float32)
        xqv = np.ascontiguousarray(x[b][s:T:2], dtype=np.float32)
        kvl = np.arange(P)[:, None]
        ul = np.arange(64)[None, :]
        mask = (2 * ul + s >= kvl).astype(np.float32)
        mask = np.tile(mask, (1, 2))
        in_maps.append(
            {
                "xc": xcv,
                "xq": xqv,
                "wq": wqh,
                "wk": wkh,
                "wv": wvh,
                "wc": wch,
                "wf": wfh,
                "wp": wph,
                "msk": mask.astype(bf),
            }
        )
    return in_maps


_NC_CACHE = {}


def _get_nc(T=2048):
    if T not in _NC_CACHE:
        _NC_CACHE[T] = build_nc(T=T)
    return _NC_CACHE[T]


def kernel(**inputs):
    """Full transformer block on 8 NeuronCores. Takes/returns full numpy arrays."""
    from concourse.bass_utils import run_bass_kernel_spmd

    x = np.asarray(inputs["x"], dtype=np.float32)
    B, T, C_ = x.shape
    nc = _get_nc(T=T)
    in_maps = stage_inputs(
        x,
        np.asarray(inputs["c_attn_w"], dtype=np.float32),
        np.asarray(inputs["c_proj_w"], dtype=np.float32),
        np.asarray(inputs["fc_w"], dtype=np.float32),
        np.asarray(inputs["proj_w"], dtype=np.float32),
        np.asarray(inputs["ln1_g"], dtype=np.float32),
        np.asarray(inputs["ln2_g"], dtype=np.float32),
        T=T,
        n_cores=8,
    )
    res = run_bass_kernel_spmd(nc, in_maps, list(range(8)))
    out = np.empty((B, T, C_), dtype=np.float32)
    for c in range(8):
        b, s = c // 2, c % 2
        out[b, s::2, :] = res.results[c]["yout"]
    return out


# revision 50
# speedup vs baseline: 1.1466x; 1.0204x over previous
"""Transformer block (B=4,T=2048,C=1024,H=16) on 8 trn2 cores, zero-communication.

Split: core c -> sequence b=c//2, token parity s=c%2. Each core computes the
full block output for its 1024 strided query tokens (positions s, s+2, ...),
recomputing LN1+K/V for the whole 2048-token context locally (no collectives).

Pipeline (v2): phases are software-pipelined to keep TensorE (and the HAM
clock) saturated end to end:
  front: LN1 tiles interleaved with kT / vA(oj=0) / qT(m=0) projections
  m=0:   attention q-cols 0..511 for all head-pairs; fillers = vA(oj=1),
         qT(m=1)
  m=1:   attention q-cols 512..1023; fillers = c_proj + residual for token
         tiles 0..3
  post:  c_proj qt 4..7, LN2 -> mT, then MLP in 4 f-quarters with
         double-buffered wf/wp streaming; proj accumulates into x2 in SBUF.

PSUM (8 banks): sc 2x[128,1024] (4) | av 2x[*,512] (2) | bc 1 (1) | fill 1 (1).
Softmax normalize: avp -> SBUF early-release copy, denom row broadcast by
matmul, reciprocal_approx_fast, tensor_mul.
"""
import sys

sys.path.insert(0, "/opt/trn_rl_repo")

import numpy as np
import ml_dtypes

import concourse.bass as bass
import concourse.mybir as mybir
import concourse.tile as tile
from concourse import bacc
from concourse.masks import make_identity

FP32 = mybir.dt.float32
BF16 = mybir.dt.bfloat16
AF = mybir.ActivationFunctionType
ALU = mybir.AluOpType

C = 1024
H = 16
HS = 64
FF = 4096
LN_EPS = 1e-5
P = 128


def build_nc(T=2048):
    own = T // 2          # query tokens per core
    NKV = T // P          # kv token tiles (16)
    QM = min(512, own)    # q-macro width
    NM = own // QM        # q macros (2)
    NQT = own // P        # own-token tiles (8)
    NCT = C // P          # feature tiles (8)
    NTM = T // 512        # kv 512-macros (4)

    nc = bacc.Bacc(None, target_bir_lowering=False, debug=False)

    xc = nc.dram_tensor("xc", [T, C], FP32, kind="ExternalInput")
    xq = nc.dram_tensor("xq", [own, C], FP32, kind="ExternalInput")
    wq = nc.dram_tensor("wq", [C, C], BF16, kind="ExternalInput")
    wk = nc.dram_tensor("wk", [C, C], BF16, kind="ExternalInput")
    wv = nc.dram_tensor("wv", [C, C], BF16, kind="ExternalInput")
    wc = nc.dram_tensor("wc", [C, C], BF16, kind="ExternalInput")
    wf = nc.dram_tensor("wf", [C, FF], BF16, kind="ExternalInput")
    wp = nc.dram_tensor("wp", [FF, C], BF16, kind="ExternalInput")
    msk = nc.dram_tensor("msk", [P, 2 * 64], BF16, kind="ExternalInput")
    yout = nc.dram_tensor("yout", [own, C], FP32, kind="ExternalOutput")

    with tile.TileContext(nc) as tc:
        import contextlib

        with contextlib.ExitStack() as ctx:
            const = ctx.enter_context(tc.tile_pool(name="const", bufs=1))
            xin = ctx.enter_context(tc.tile_pool(name="xin", bufs=2))
            lnp = ctx.enter_context(tc.tile_pool(name="lnp", bufs=3))
            nbp = ctx.enter_context(tc.tile_pool(name="nbp", bufs=2))
            g1 = ctx.enter_context(tc.tile_pool(name="g1", bufs=NCT))   # nT -> hq
            g2 = ctx.enter_context(tc.tile_pool(name="g2", bufs=NCT))   # kT -> wf/wp
            g3 = ctx.enter_context(tc.tile_pool(name="g3", bufs=NCT))   # nqT -> x2
            vap = ctx.enter_context(tc.tile_pool(name="vap", bufs=NKV))  # vA -> mT
            qwp = ctx.enter_context(tc.tile_pool(name="qwp", bufs=NCT))  # qT
            wcp = ctx.enter_context(tc.tile_pool(name="wcp", bufs=NCT))  # wc halves
            atp = ctx.enter_context(tc.tile_pool(name="atp", bufs=NCT))  # attnT
            exq = ctx.enter_context(tc.tile_pool(name="exq", bufs=3))    # ex + xr
            smp = ctx.enter_context(tc.tile_pool(name="smp", bufs=3))    # softmax
            wsp = ctx.enter_context(tc.tile_pool(name="wsp", bufs=8))    # wq/wk blk
            wvp = ctx.enter_context(tc.tile_pool(name="wvp", bufs=8))   # wv halves

            ps = ctx.enter_context(tc.tile_pool(name="ps", bufs=1, space="PSUM"))

            _psn = [0, 0]

            def ps_small(name, shape=(P, 512), dtype=FP32, in_attn=False):
                """Rotating one-bank psum tile.

                Outside attention: cycles av/bc/fill (ring of 4 incl. av's 2
                slots). Inside attention the av slots are HELD by the live
                softmax accumulators — allocating av there deadlocks the PE —
                so fillers cycle bc/fill only."""
                if in_attn:
                    tag = ("bc", "fill")[_psn[1] % 2]
                    _psn[1] += 1
                else:
                    tag = ("av", "bc", "fill")[_psn[0] % 3]
                    _psn[0] += 1
                bufs = {"av": 2, "bc": 1, "fill": 1}[tag]
                return ps.tile(list(shape), dtype, tag=tag, bufs=bufs, name=name)

            ident = const.tile([P, P], BF16)
            make_identity(nc, ident)
            ones65 = const.tile([65, 64], BF16)
            nc.vector.memset(ones65[64:65, :], 1.0)

            epst = const.tile([P, 1], FP32)
            nc.vector.memset(epst, LN_EPS)
            maskt = const.tile([P, 2 * 64], BF16)
            nc.sync.dma_start(maskt[:], msk[:])

            # PE warmup: HAM clock ramp while the first LN chunks stream in.
            for wi in range(48):
                wps = ps.tile([P, P], BF16, tag="sc", bufs=2, name=f"warm{wi}")
                nc.tensor.transpose(wps[:], ident[:], ident[:])

            _ceng = [0]

            def trans_copy(dst, src):
                """PSUM->SBUF [128,128] copy, alternated scalar/vector
                (GpSimd cannot read PSUM)."""
                k = _ceng[0] % 2
                _ceng[0] += 1
                if k == 0:
                    nc.scalar.activation(dst, src, AF.Copy)
                else:
                    nc.vector.tensor_copy(dst, src)

            def layer_norm_to_bf16(xt, out_bf, uid):
                """xt [128, C] f32 -> out_bf [128, C] bf16 (normalized).

                Stats on VectorE; the big apply on ScalarE via
                Identity(x*rs + (-mu*rs))."""
                stats = lnp.tile([P, 2, 6], FP32, tag="stats", name=f"st{uid}")
                nc.vector.bn_stats(stats[:, 0, :], xt[:, 0:512])
                nc.vector.bn_stats(stats[:, 1, :], xt[:, 512:1024])
                mv = lnp.tile([P, 2], FP32, tag="mv", name=f"mv{uid}")
                nc.vector.bn_aggr(mv[:], stats[:])
                sd = lnp.tile([P, 1], FP32, tag="sd", name=f"sd{uid}")
                nc.scalar.activation(sd[:], mv[:, 1:2], AF.Sqrt, bias=epst[:])
                rs = lnp.tile([P, 1], FP32, tag="rs", name=f"rs{uid}")
                nc.vector.reciprocal(rs[:], sd[:])
                nmr = lnp.tile([P, 1], FP32, tag="nmr", name=f"nmr{uid}")
                nc.vector.scalar_tensor_tensor(
                    out=nmr[:],
                    in0=mv[:, 0:1],
                    scalar=-1.0,
                    in1=rs[:],
                    op0=ALU.mult,
                    op1=ALU.mult,
                )
                nc.scalar.activation(
                    out_bf[:], xt[:], AF.Identity, bias=nmr[:], scale=rs[:]
                )

            nT = [g1.tile([P, T], BF16, tag="g1", name=f"nT{i}") for i in range(NCT)]
            nqT = [g3.tile([P, own], BF16, tag="g3", name=f"nqT{i}") for i in range(NCT)]
            kT = [g2.tile([P, T], BF16, tag="g2", name=f"kT{i}") for i in range(NCT)]
            qT = [qwp.tile([P, own], BF16, tag="qw", name=f"qT{i}") for i in range(NCT)]
            vA = []
            for tt in range(NKV):
                v = vap.tile([P, H * 65], BF16, tag="va", name=f"vA{tt}")
                v3 = v.rearrange("p (h k) -> p h k", k=65)
                nc.vector.memset(v3[:, :, 64:65], 1.0)
                vA.append(v)

            _ln_nb = {}

            def ln_load(kt, src, pfx):
                """DMA + LN (Vector/Scalar only, no PE) -> staged nb tile."""
                xt = xin.tile([P, C], FP32, tag="xt", name=f"x{pfx}{kt}")
                nc.sync.dma_start(xt[:], src[P * kt : P * (kt + 1), :])
                nb = nbp.tile([P, C], BF16, tag="nb", name=f"nb{pfx}{kt}")
                layer_norm_to_bf16(xt, nb, f"{pfx}{kt}")
                _ln_nb[(pfx, kt)] = nb

            def ln_trans(kt, dstT, pfx, in_attn=False):
                """8 transposes of a staged nb tile into feature-major dstT."""
                nb = _ln_nb.pop((pfx, kt))
                for ct in range(NCT):
                    if in_attn:
                        pst = ps_small(f"tr{pfx}{kt}_{ct}", (P, P), BF16, True)
                    else:
                        pst = ps.tile(
                            [P, P], BF16, tag="sc", bufs=2, name=f"tr{pfx}{kt}_{ct}"
                        )
                    nc.tensor.transpose(pst[:], nb[:, P * ct : P * (ct + 1)], ident[:])
                    if in_attn:
                        nc.vector.tensor_copy(dstT[ct][:, P * kt : P * (kt + 1)], pst[:])
                    else:
                        trans_copy(dstT[ct][:, P * kt : P * (kt + 1)], pst[:])

            def keepalive(n, uid):
                """Dummy PE transposes — keep the HAM clock up through
                PE-sparse stretches (results unread)."""
                for i in range(n):
                    wps = ps.tile([P, P], BF16, tag="sc", bufs=2, name=f"ka{uid}_{i}")
                    nc.tensor.transpose(wps[:], ident[:], ident[:])

            def emit_ln1(kt, in_attn=False):
                ln_load(kt, xc, "a")
                ln_trans(kt, nT, "a", in_attn)

            wkb_cache = {}

            def load_wk(ot):
                blks = []
                for ci in range(NCT):
                    w = wsp.tile([P, P], BF16, tag="qk", name=f"wk{ot}_{ci}")
                    nc.sync.dma_start(
                        w[:], wk[P * ci : P * (ci + 1), P * ot : P * (ot + 1)]
                    )
                    blks.append(w)
                wkb_cache.clear()
                wkb_cache[ot] = blks

            def emit_kT(ot, tm, in_attn=False):
                if ot not in wkb_cache:
                    load_wk(ot)
                wkb = wkb_cache[ot]
                p = ps_small(f"kps{ot}_{tm}", in_attn=in_attn)
                for ci in range(NCT):
                    nc.tensor.matmul(
                        p[:],
                        wkb[ci][:],
                        nT[ci][:, 512 * tm : 512 * (tm + 1)],
                        start=(ci == 0),
                        stop=(ci == NCT - 1),
                    )
                if in_attn:
                    nc.vector.tensor_copy(kT[ot][:, 512 * tm : 512 * (tm + 1)], p[:])
                else:
                    nc.scalar.activation(
                        kT[ot][:, 512 * tm : 512 * (tm + 1)], p[:], AF.Copy
                    )

            wvhs = {}

            def load_wv(oj):
                wvh = []
                for ci in range(NCT):
                    w = wvp.tile([P, 512], BF16, tag="v", name=f"wv{oj}_{ci}")
                    nc.sync.dma_start(
                        w[:], wv[P * ci : P * (ci + 1), 512 * oj : 512 * (oj + 1)]
                    )
                    wvh.append(w)
                wvhs[oj] = wvh

            def emit_v(oj, tt, in_attn):
                p = ps_small(f"vps{oj}_{tt}", in_attn=in_attn)
                for ci in range(NCT):
                    nc.tensor.matmul(
                        p[:],
                        nT[ci][:, P * tt : P * (tt + 1)],
                        wvhs[oj][ci][:],
                        start=(ci == 0),
                        stop=(ci == NCT - 1),
                    )
                v3 = vA[tt].rearrange("p (h k) -> p h k", k=65)
                ps3 = p.rearrange("p (h k) -> p h k", k=64)
                if in_attn:
                    nc.vector.tensor_copy(v3[:, 8 * oj : 8 * (oj + 1), 0:64], ps3[:])
                else:
                    nc.scalar.activation(
                        v3[:, 8 * oj : 8 * (oj + 1), 0:64], ps3[:], AF.Copy
                    )

            wqb_cache = {}

            def load_wq(ot, m):
                wqb = []
                for ci in range(NCT):
                    w = wsp.tile([P, P], BF16, tag="qk", name=f"wq{ot}_{m}_{ci}")
                    nc.sync.dma_start(
                        w[:], wq[P * ci : P * (ci + 1), P * ot : P * (ot + 1)]
                    )
                    wqb.append(w)
                wqb_cache.clear()
                wqb_cache[ot] = wqb

            def emit_qT(ot, m, in_attn):
                if ot not in wqb_cache:
                    load_wq(ot, m)
                wqb = wqb_cache[ot]
                p = ps_small(f"qps{ot}_{m}", (P, QM), in_attn=in_attn)
                for ci in range(NCT):
                    nc.tensor.matmul(
                        p[:],
                        wqb[ci][:],
                        nqT[ci][:, QM * m : QM * (m + 1)],
                        start=(ci == 0),
                        stop=(ci == NCT - 1),
                    )
                if in_attn:
                    nc.vector.tensor_copy(qT[ot][:, QM * m : QM * (m + 1)], p[:])
                else:
                    nc.scalar.activation(
                        qT[ot][:, QM * m : QM * (m + 1)], p[:], AF.Copy
                    )

            _sc = nc.enter_named_scope("ph_front", False)[0]
            # ---- front (minimal for m=0 start): LN1 tiles 0-7, A2 0-3,
            # kT[*] first-half context, vA0 0-7, qT m=0. The rest streams in
            # as m=0 fillers.
            load_wv(0)
            for kt in range(4):
                ln_load(kt, xc, "a")
                keepalive(20, f"f{kt}")
                ln_trans(kt, nT, "a")
            emit_kT(0, 0)
            emit_v(0, 0, False)
            emit_v(0, 1, False)
            for kt in range(4, 8):
                ln_load(kt, xc, "a")
                keepalive(12, f"f{kt}")
                ln_trans(kt, nT, "a")
            emit_kT(0, 1)
            for tt in range(2, 6):
                emit_v(0, tt, False)
            emit_kT(1, 0)
            emit_kT(1, 1)
            emit_v(0, 6, False)
            emit_v(0, 7, False)
            for qt in range(0, 2):
                ln_load(qt, xq, "q")
                ln_trans(qt, nqT, "q")
            for ot in (2, 3):
                emit_kT(ot, 0)
                emit_kT(ot, 1)
            for qt in range(2, 4):
                ln_load(qt, xq, "q")
                ln_trans(qt, nqT, "q")
            for ot in (4, 5, 6, 7):
                emit_kT(ot, 0)
                emit_kT(ot, 1)
            for ot in range(NCT):
                emit_qT(ot, 0, False)

            nc.leave_named_scope("ph_front", _sc, False)

            # ---- attention: m outer, hp inner ------------------------------
            attnT = [
                atp.tile([P, own], BF16, tag="at", name=f"attnT{i}") for i in range(NCT)
            ]
            x2 = [None] * NQT
            wc_blks = {}

            def load_wc(oj, phase):
                blks = []
                for ci in range(NCT):
                    w = wcp.tile([P, 512], BF16, tag="wc", name=f"wc{phase}_{oj}_{ci}")
                    nc.sync.dma_start(
                        w[:], wc[P * ci : P * (ci + 1), 512 * oj : 512 * (oj + 1)]
                    )
                    blks.append(w)
                wc_blks[oj] = blks

            def emit_cproj(qt, oj, in_attn=False):
                if x2[qt] is None:
                    x2[qt] = g3.tile([P, C], FP32, tag="g3", name=f"x2_{qt}")
                xr = exq.tile([P, 512], FP32, tag="xr", bufs=1, name=f"xr{qt}_{oj}")
                nc.sync.dma_start(
                    xr[:], xq[P * qt : P * (qt + 1), 512 * oj : 512 * (oj + 1)]
                )
                p = ps_small(f"cps{qt}_{oj}", in_attn=in_attn)
                for ci in range(NCT):
                    nc.tensor.matmul(
                        p[:],
                        attnT[ci][:, P * qt : P * (qt + 1)],
                        wc_blks[oj][ci][:],
                        start=(ci == 0),
                        stop=(ci == NCT - 1),
                    )
                nc.vector.tensor_add(
                    x2[qt][:, 512 * oj : 512 * (oj + 1)], p[:], xr[:]
                )

            # filler queues, popped INSIDE the j-loop (~2us granularity) so PE
            # duty stays high through scalar-bound attention blocks and the
            # HAM clock never drops. Ordering respects data deps.
            def T(f, *a):
                return lambda: f(*a)

            # NOTE ordering: ALL vA oj=0 tiles must be emitted before
            # load_wv(1) — the wv oj=1 DMAs reuse the wvp ring slots whose
            # release requires every vA0 matmul, and the sync engine is
            # in-order (a late vA0 dep would deadlock the DMA queue).
            # q_m0a must be fully emitted before the hp=4 block of m=0 (its
            # AV matmuls read vA oj=1); q_m0b just needs to land within m=0.
            q_m0a = [
                T(ln_load, 8, xc, "a"), T(ln_load, 9, xc, "a"),
                T(ln_trans, 8, nT, "a", True), T(ln_trans, 9, nT, "a", True),
                T(emit_v, 0, 8, True), T(ln_load, 10, xc, "a"),
                T(emit_v, 0, 9, True), T(ln_load, 11, xc, "a"),
                T(ln_trans, 10, nT, "a", True), T(ln_trans, 11, nT, "a", True),
                T(emit_v, 0, 10, True), T(ln_load, 12, xc, "a"),
                T(emit_v, 0, 11, True), T(ln_load, 13, xc, "a"),
                T(ln_trans, 12, nT, "a", True), T(ln_trans, 13, nT, "a", True),
                T(emit_v, 0, 12, True), T(ln_load, 14, xc, "a"),
                T(emit_v, 0, 13, True), T(ln_load, 15, xc, "a"),
                T(ln_trans, 14, nT, "a", True), T(ln_trans, 15, nT, "a", True),
                T(emit_v, 0, 14, True), T(emit_v, 0, 15, True),
                T(load_wv, 1),
            ]
            for tt in range(NKV):
                q_m0a.append(T(emit_v, 1, tt, True))
            q_m0b = []
            for ot in range(NCT):
                q_m0b.append(T(load_wk, ot))
                q_m0b.append(T(emit_kT, ot, 2, True))
                q_m0b.append(T(emit_kT, ot, 3, True))
            for qt in (4, 5):
                q_m0b.append(T(ln_load, qt, xq, "q"))
            for qt in (4, 5):
                q_m0b.append(T(ln_trans, qt, nqT, "q", True))
            for qt in (6, 7):
                q_m0b.append(T(ln_load, qt, xq, "q"))
            for qt in (6, 7):
                q_m0b.append(T(ln_trans, qt, nqT, "q", True))
            for ot in range(NCT):
                q_m0b.append(T(load_wq, ot, 1))
                q_m0b.append(T(emit_qT, ot, 1, True))
            q_m0 = q_m0a + q_m0b
            q_m0a_set = set(q_m0a)

            q_m1 = [T(load_wc, 0, "m1")]
            for qt in range(4):
                q_m1.append(T(emit_cproj, qt, 0, True))
            q_m1.append(T(load_wc, 1, "m1"))
            for qt in range(4):
                q_m1.append(T(emit_cproj, qt, 1, True))


            mask3 = maskt.rearrange("p (r k) -> p r k", r=2)

            def emit_normalize(hp, r, m, avp_r):
                h = 2 * hp + r
                av_s = smp.tile([65, QM], BF16, tag="avs", bufs=2, name=f"avs{h}_{m}")
                with nc.allow_low_precision(reason="attn out + denom to bf16"):
                    nc.vector.tensor_copy(av_s[:], avp_r[:])
                bcp = ps.tile([64, QM], FP32, tag="bc", bufs=1, name=f"bc{h}_{m}")
                nc.tensor.matmul(
                    bcp[:], ones65[64:65, :], av_s[64:65, :], start=True, stop=True
                )
                bcs = smp.tile([64, QM], FP32, tag="bcs", bufs=2, name=f"bcs{h}_{m}")
                nc.vector.reciprocal_approx_fast(bcs[:], bcp[:])
                nc.vector.tensor_mul(
                    attnT[hp][64 * r : 64 * r + 64, QM * m : QM * (m + 1)],
                    av_s[0:64, :],
                    bcs[:],
                )

            _sc = nc.enter_named_scope("ph_attn", False)[0]
            for m in range(NM):
                jmax0 = 2 * QM * (m + 1) // P
                fq = q_m0 if m == 0 else q_m1
                steps_left = [8 * jmax0]

                def pump():
                    """Pop filler thunks, pacing the queue across the m-block."""
                    if not fq:
                        return
                    n = max(1, -(-len(fq) // max(1, steps_left[0])))
                    for _ in range(min(n, 2)):
                        if fq:
                            fq.pop(0)()
                    steps_left[0] -= 1

                for hp in range(H // 2):
                    if m == 0 and hp == 4:
                        # hp>=4 AV matmuls read vA oj=1 — force q_m0a flushed
                        while fq and fq[0] in q_m0a_set:
                            fq.pop(0)()
                    avp = [
                        ps.tile([65, QM], FP32, tag="av", bufs=2, name=f"av{hp}_{m}_{r}")
                        for r in range(2)
                    ]
                    exs = {}

                    def emit_av(j, r):
                        ex, w0 = exs[(j, r)]
                        nc.tensor.matmul(
                            avp[r][:, w0:QM],
                            vA[j][:, 65 * (2 * hp + r) : 65 * (2 * hp + r) + 65],
                            ex[:, QM * r + w0 : QM * (r + 1)],
                            start=(j == 0),
                            stop=(j == jmax0 - 1),
                        )
                        if r == 1:
                            del exs[(j, 0)], exs[(j, 1)]

                    for j in range(jmax0):
                        wq_ = max(0, (P * j - 2 * QM * m) // 2)
                        sc = ps.tile(
                            [P, 2 * QM], FP32, tag="sc", bufs=2,
                            name=f"sc{hp}_{m}_{j}",
                        )
                        for r in range(2):
                            nc.tensor.matmul(
                                sc[:, QM * r + wq_ : QM * (r + 1)],
                                kT[hp][64 * r : 64 * r + 64, P * j : P * (j + 1)],
                                qT[hp][
                                    64 * r : 64 * r + 64, QM * m + wq_ : QM * (m + 1)
                                ],
                                start=True,
                                stop=True,
                            )
                        ex = exq.tile(
                            [P, 2 * QM], BF16, tag="ex", bufs=3,
                            name=f"ex{hp}_{m}_{j}",
                        )
                        sc3 = sc.rearrange("p (r q) -> p r q", r=2)
                        ex3 = ex.rearrange("p (r q) -> p r q", r=2)
                        nc.scalar.activation(
                            ex3[:, :, wq_:QM], sc3[:, :, wq_:QM], AF.Exp
                        )
                        if P * j >= 2 * QM * m:
                            nc.gpsimd.tensor_mul(
                                ex3[:, :, wq_ : wq_ + 64],
                                ex3[:, :, wq_ : wq_ + 64],
                                mask3[:],
                            )
                        exs[(j, 0)] = (ex, wq_)
                        exs[(j, 1)] = (ex, wq_)
                        if j >= 1:
                            emit_av(j - 1, 0)
                            emit_av(j - 1, 1)
                        if m == 0 or j % 8 == 3:
                            pump()
                    emit_av(jmax0 - 1, 0)
                    emit_av(jmax0 - 1, 1)
                    emit_normalize(hp, 0, m, avp[0])
                    emit_normalize(hp, 1, m, avp[1])
                # everything queued for this m must land before the next m
                while fq:
                    fq.pop(0)()
            nc.leave_named_scope("ph_attn", _sc, False)

            _sc = nc.enter_named_scope("ph_post", False)[0]
            # ---- post: c_proj qt4-7 + LN2 interleaved, MLP in f-quarters ---
            mT = [vap.tile([P, own], BF16, tag="va", name=f"mT{i}") for i in range(NCT)]

            def emit_ln2(qt):
                mb = nbp.tile([P, C], BF16, tag="nb", name=f"mb{qt}")
                layer_norm_to_bf16(x2[qt], mb, f"m{qt}")
                for ct in range(NCT):
                    pst = ps_small(f"mtr{qt}_{ct}", (P, P), BF16)
                    nc.tensor.transpose(pst[:], mb[:, P * ct : P * (ct + 1)], ident[:])
                    trans_copy(mT[ct][:, P * qt : P * (qt + 1)], pst[:])

            def load_wfq(fqi):
                wf4 = []
                for k in range(4):
                    w = g2.tile([P, 2048], BF16, tag="g2", name=f"wf{fqi}_{k}")
                    for half in range(2):
                        ci = 2 * k + half
                        nc.sync.dma_start(
                            w[:, 1024 * half : 1024 * (half + 1)],
                            wf[P * ci : P * (ci + 1), 1024 * fqi : 1024 * (fqi + 1)],
                        )
                    wf4.append(w)
                return wf4

            def load_wpq(fqi):
                wp4 = []
                for k in range(4):
                    w = g2.tile([P, 2048], BF16, tag="g2", name=f"wp{fqi}_{k}")
                    for half in range(2):
                        ftl = 2 * k + half
                        r0 = 1024 * fqi + P * ftl
                        nc.sync.dma_start(
                            w[:, 1024 * half : 1024 * (half + 1)], wp[r0 : r0 + P, :]
                        )
                    wp4.append(w)
                return wp4

            def emit_fc(fqi, wf4, hq, ftl, mq):
                p = ps.tile(
                    [P, QM], FP32, tag="sc", bufs=2, name=f"fps{fqi}_{ftl}_{mq}"
                )
                for ci in range(NCT):
                    nc.tensor.matmul(
                        p[:],
                        wf4[ci // 2][
                            :, 1024 * (ci % 2) + P * ftl : 1024 * (ci % 2)
                            + P * (ftl + 1)
                        ],
                        mT[ci][:, QM * mq : QM * (mq + 1)],
                        start=(ci == 0),
                        stop=(ci == NCT - 1),
                    )
                nc.scalar.activation(
                    hq[ftl][:, QM * mq : QM * (mq + 1)], p[:], AF.Gelu_apprx_tanh
                )

            def emit_proj(fqi, wp4, hq, qt):
                for oj in range(2):
                    p = ps_small(f"pps{fqi}_{qt}_{oj}")
                    for ftl in range(8):
                        nc.tensor.matmul(
                            p[:],
                            hq[ftl][:, P * qt : P * (qt + 1)],
                            wp4[ftl // 2][
                                :, 1024 * (ftl % 2) + 512 * oj : 1024 * (ftl % 2)
                                + 512 * (oj + 1)
                            ],
                            start=(ftl == 0),
                            stop=(ftl == 7),
                        )
                    nc.vector.tensor_add(
                        x2[qt][:, 512 * oj : 512 * (oj + 1)],
                        p[:],
                        x2[qt][:, 512 * oj : 512 * (oj + 1)],
                    )

            # start of post: c_proj(qt4-7) rides over LN2's vector work; the
            # first fc chains overlap LN2 of qt4-7.
            load_wc(0, "p")
            for qt in range(4, 8):
                emit_cproj(qt, 0)
                emit_ln2(qt - 4)
            load_wc(1, "p")
            wf4_0 = load_wfq(0)
            wp4_0 = load_wpq(0)
            for qt in range(4, 8):
                emit_cproj(qt, 1)
            hq0 = [g1.tile([P, own], BF16, tag="g1", name=f"hq0_{i}") for i in range(8)]
            for ftl in range(8):
                emit_fc(0, wf4_0, hq0, ftl, 0)
                if ftl < 4:
                    emit_ln2(4 + ftl)
            for ftl in range(8):
                emit_fc(0, wf4_0, hq0, ftl, 1)
            for qt in range(NQT):
                emit_proj(0, wp4_0, hq0, qt)

            for fqi in range(1, 4):
                wf4 = load_wfq(fqi)
                wp4 = load_wpq(fqi)
                hq = [
                    g1.tile([P, own], BF16, tag="g1", name=f"hq{fqi}_{i}")
                    for i in range(8)
                ]
                for ftl in range(8):
                    for mq in range(2):
                        emit_fc(fqi, wf4, hq, ftl, mq)
                for qt in range(NQT):
                    emit_proj(fqi, wp4, hq, qt)
            for qt in range(NQT):
                nc.sync.dma_start(yout[P * qt : P * (qt + 1), :], x2[qt][:])
            nc.leave_named_scope("ph_post", _sc, False)

    nc.compile()
    return nc


def stage_inputs(x, c_attn_w, c_proj_w, fc_w, proj_w, ln1_g, ln2_g, T=2048, n_cores=8):
    """Host-side prep: per-core input maps. x: (B, T, C) f32."""
    bf = ml_dtypes.bfloat16
    g1w = c_attn_w * ln1_g[:, None]
    wqh = np.ascontiguousarray((g1w[:, 0:C] * 0.125).astype(bf))
    wkh = np.ascontiguousarray(g1w[:, C : 2 * C].astype(bf))
    wvh = np.ascontiguousarray(g1w[:, 2 * C : 3 * C].astype(bf))
    wch = np.ascontiguousarray(c_proj_w.astype(bf))
    wfh = np.ascontiguousarray((fc_w * ln2_g[:, None]).astype(bf))
    wph = np.ascontiguousarray(proj_w.astype(bf))
    in_maps = []
    for c in range(n_cores):
        b, s = c // 2, c % 2
        xcv = np.ascontiguousarray(x[b][:T], dtype=np.float32)
        xqv = np.ascontiguousarray(x[b][s:T:2], dtype=np.float32)
        kvl = np.arange(P)[:, None]
        ul = np.arange(64)[None, :]
        mask = (2 * ul + s >= kvl).astype(np.float32)
        mask = np.tile(mask, (1, 2))
        in_maps.append(
            {
                "xc": xcv,
                "xq": xqv,
                "wq": wqh,
                "wk": wkh,
                "wv": wvh,
                "wc": wch,
                "wf": wfh,
                "wp": wph,
                "msk": mask.astype(bf),
            }
        )
    return in_maps


_NC_CACHE = {}


def _get_nc(T=2048):
    if T not in _NC_CACHE:
        _NC_CACHE[T] = build_nc(T=T)
    return _NC_CACHE[T]


def kernel(**inputs):
    """Full transformer block on 8 NeuronCores. Takes/returns full numpy arrays."""
    from concourse.bass_utils import run_bass_kernel_spmd

    x = np.asarray(inputs["x"], dtype=np.float32)
    B, T, C_ = x.shape
    nc = _get_nc(T=T)
    in_maps = stage_inputs(
        x,
        np.asarray(inputs["c_attn_w"], dtype=np.float32),
        np.asarray(inputs["c_proj_w"], dtype=np.float32),
        np.asarray(inputs["fc_w"], dtype=np.float32),
        np.asarray(inputs["proj_w"], dtype=np.float32),
        np.asarray(inputs["ln1_g"], dtype=np.float32),
        np.asarray(inputs["ln2_g"], dtype=np.float32),
        T=T,
        n_cores=8,
    )
    res = run_bass_kernel_spmd(nc, in_maps, list(range(8)))
    out = np.empty((B, T, C_), dtype=np.float32)
    for c in range(8):
        b, s = c // 2, c % 2
        out[b, s::2, :] = res.results[c]["yout"]
    return out


# revision 52
# speedup vs baseline: 1.1585x; 1.0103x over previous
"""Transformer block (B=4,T=2048,C=1024,H=16) on 8 trn2 cores, zero-communication.

Split: core c -> sequence b=c//2, token parity s=c%2. Each core computes the
full block output for its 1024 strided query tokens (positions s, s+2, ...),
recomputing LN1+K/V for the whole 2048-token context locally (no collectives).

Pipeline (v2): phases are software-pipelined to keep TensorE (and the HAM
clock) saturated end to end:
  front: LN1 tiles interleaved with kT / vA(oj=0) / qT(m=0) projections
  m=0:   attention q-cols 0..511 for all head-pairs; fillers = vA(oj=1),
         qT(m=1)
  m=1:   attention q-cols 512..1023; fillers = c_proj + residual for token
         tiles 0..3
  post:  c_proj qt 4..7, LN2 -> mT, then MLP in 4 f-quarters with
         double-buffered wf/wp streaming; proj accumulates into x2 in SBUF.

PSUM (8 banks): sc 2x[128,1024] (4) | av 2x[*,512] (2) | bc 1 (1) | fill 1 (1).
Softmax normalize: avp -> SBUF early-release copy, denom row broadcast by
matmul, reciprocal_approx_fast, tensor_mul.
"""
import sys

sys.path.insert(0, "/opt/trn_rl_repo")

import numpy as np
import ml_dtypes

import concourse.bass as bass
import concourse.mybir as mybir
import concourse.tile as tile
from concourse import bacc
from concourse.masks import make_identity

FP32 = mybir.dt.float32
BF16 = mybir.dt.bfloat16
AF = mybir.ActivationFunctionType
ALU = mybir.AluOpType

C = 1024
H = 16
HS = 64
FF = 4096
LN_EPS = 1e-5
P = 128


def build_nc(T=2048):
    own = T // 2          # query tokens per core
    NKV = T // P          # kv token tiles (16)
    QM = min(512, own)    # q-macro width
    NM = own // QM        # q macros (2)
    NQT = own // P        # own-token tiles (8)
    NCT = C // P          # feature tiles (8)
    NTM = T // 512        # kv 512-macros (4)

    nc = bacc.Bacc(None, target_bir_lowering=False, debug=False)

    xc = nc.dram_tensor("xc", [T, C], FP32, kind="ExternalInput")
    xq = nc.dram_tensor("xq", [own, C], FP32, kind="ExternalInput")
    wq = nc.dram_tensor("wq", [C, C], BF16, kind="ExternalInput")
    wk = nc.dram_tensor("wk", [C, C], BF16, kind="ExternalInput")
    wv = nc.dram_tensor("wv", [C, C], BF16, kind="ExternalInput")
    wc = nc.dram_tensor("wc", [C, C], BF16, kind="ExternalInput")
    wf = nc.dram_tensor("wf", [C, FF], BF16, kind="ExternalInput")
    wp = nc.dram_tensor("wp", [FF, C], BF16, kind="ExternalInput")
    msk = nc.dram_tensor("msk", [P, 2 * 64], BF16, kind="ExternalInput")
    yout = nc.dram_tensor("yout", [own, C], FP32, kind="ExternalOutput")

    with tile.TileContext(nc) as tc:
        import contextlib

        with contextlib.ExitStack() as ctx:
            const = ctx.enter_context(tc.tile_pool(name="const", bufs=1))
            xin = ctx.enter_context(tc.tile_pool(name="xin", bufs=2))
            lnp = ctx.enter_context(tc.tile_pool(name="lnp", bufs=3))
            nbp = ctx.enter_context(tc.tile_pool(name="nbp", bufs=2))
            g1 = ctx.enter_context(tc.tile_pool(name="g1", bufs=NCT))   # nT -> hq
            g2 = ctx.enter_context(tc.tile_pool(name="g2", bufs=NCT))   # kT -> wf/wp
            g3 = ctx.enter_context(tc.tile_pool(name="g3", bufs=NCT))   # nqT -> x2
            vap = ctx.enter_context(tc.tile_pool(name="vap", bufs=NKV))  # vA -> mT
            qwp = ctx.enter_context(tc.tile_pool(name="qwp", bufs=NCT))  # qT
            wcp = ctx.enter_context(tc.tile_pool(name="wcp", bufs=NCT))  # wc halves
            atp = ctx.enter_context(tc.tile_pool(name="atp", bufs=NCT))  # attnT
            exq = ctx.enter_context(tc.tile_pool(name="exq", bufs=3))    # ex + xr
            smp = ctx.enter_context(tc.tile_pool(name="smp", bufs=3))    # softmax
            wsp = ctx.enter_context(tc.tile_pool(name="wsp", bufs=8))    # wq/wk blk
            wvp = ctx.enter_context(tc.tile_pool(name="wvp", bufs=8))   # wv halves

            ps = ctx.enter_context(tc.tile_pool(name="ps", bufs=1, space="PSUM"))

            _psn = [0, 0]

            def ps_small(name, shape=(P, 512), dtype=FP32, in_attn=False):
                """Rotating one-bank psum tile.

                Outside attention: cycles av/bc/fill (ring of 4 incl. av's 2
                slots). Inside attention the av slots are HELD by the live
                softmax accumulators — allocating av there deadlocks the PE —
                so fillers cycle bc/fill only."""
                if in_attn:
                    tag = ("bc", "fill")[_psn[1] % 2]
                    _psn[1] += 1
                else:
                    tag = ("av", "bc", "fill")[_psn[0] % 3]
                    _psn[0] += 1
                bufs = {"av": 2, "bc": 1, "fill": 1}[tag]
                return ps.tile(list(shape), dtype, tag=tag, bufs=bufs, name=name)

            ident = const.tile([P, P], BF16)
            make_identity(nc, ident)
            ones65 = const.tile([65, 64], BF16)
            nc.vector.memset(ones65[64:65, :], 1.0)

            epst = const.tile([P, 1], FP32)
            nc.vector.memset(epst, LN_EPS)
            maskt = const.tile([P, 2 * 64], BF16)
            nc.sync.dma_start(maskt[:], msk[:])

            # PE warmup: HAM clock ramp while the first LN chunks stream in.
            for wi in range(48):
                wps = ps.tile([P, P], BF16, tag="sc", bufs=2, name=f"warm{wi}")
                nc.tensor.transpose(wps[:], ident[:], ident[:])

            _ceng = [0]

            def trans_copy(dst, src):
                """PSUM->SBUF [128,128] copy, alternated scalar/vector
                (GpSimd cannot read PSUM)."""
                k = _ceng[0] % 2
                _ceng[0] += 1
                if k == 0:
                    nc.scalar.activation(dst, src, AF.Copy)
                else:
                    nc.vector.tensor_copy(dst, src)

            def layer_norm_to_bf16(xt, out_bf, uid):
                """xt [128, C] f32 -> out_bf [128, C] bf16 (normalized).

                Stats on VectorE; the big apply on ScalarE via
                Identity(x*rs + (-mu*rs))."""
                stats = lnp.tile([P, 2, 6], FP32, tag="stats", name=f"st{uid}")
                nc.vector.bn_stats(stats[:, 0, :], xt[:, 0:512])
                nc.vector.bn_stats(stats[:, 1, :], xt[:, 512:1024])
                mv = lnp.tile([P, 2], FP32, tag="mv", name=f"mv{uid}")
                nc.vector.bn_aggr(mv[:], stats[:])
                sd = lnp.tile([P, 1], FP32, tag="sd", name=f"sd{uid}")
                nc.scalar.activation(sd[:], mv[:, 1:2], AF.Sqrt, bias=epst[:])
                rs = lnp.tile([P, 1], FP32, tag="rs", name=f"rs{uid}")
                nc.vector.reciprocal(rs[:], sd[:])
                nmr = lnp.tile([P, 1], FP32, tag="nmr", name=f"nmr{uid}")
                nc.vector.scalar_tensor_tensor(
                    out=nmr[:],
                    in0=mv[:, 0:1],
                    scalar=-1.0,
                    in1=rs[:],
                    op0=ALU.mult,
                    op1=ALU.mult,
                )
                nc.scalar.activation(
                    out_bf[:], xt[:], AF.Identity, bias=nmr[:], scale=rs[:]
                )

            nT = [g1.tile([P, T], BF16, tag="g1", name=f"nT{i}") for i in range(NCT)]
            nqT = [g3.tile([P, own], BF16, tag="g3", name=f"nqT{i}") for i in range(NCT)]
            kT = [g2.tile([P, T], BF16, tag="g2", name=f"kT{i}") for i in range(NCT)]
            qT = [qwp.tile([P, own], BF16, tag="qw", name=f"qT{i}") for i in range(NCT)]
            vA = []
            for tt in range(NKV):
                v = vap.tile([P, H * 65], BF16, tag="va", name=f"vA{tt}")
                v3 = v.rearrange("p (h k) -> p h k", k=65)
                nc.vector.memset(v3[:, :, 64:65], 1.0)
                vA.append(v)

            _ln_nb = {}

            def ln_load(kt, src, pfx):
                """DMA + LN (Vector/Scalar only, no PE) -> staged nb tile."""
                xt = xin.tile([P, C], FP32, tag="xt", name=f"x{pfx}{kt}")
                nc.sync.dma_start(xt[:], src[P * kt : P * (kt + 1), :])
                nb = nbp.tile([P, C], BF16, tag="nb", name=f"nb{pfx}{kt}")
                layer_norm_to_bf16(xt, nb, f"{pfx}{kt}")
                _ln_nb[(pfx, kt)] = nb

            def ln_trans(kt, dstT, pfx, in_attn=False):
                """8 transposes of a staged nb tile into feature-major dstT."""
                nb = _ln_nb.pop((pfx, kt))
                for ct in range(NCT):
                    if in_attn:
                        pst = ps_small(f"tr{pfx}{kt}_{ct}", (P, P), BF16, True)
                    else:
                        pst = ps.tile(
                            [P, P], BF16, tag="sc", bufs=2, name=f"tr{pfx}{kt}_{ct}"
                        )
                    nc.tensor.transpose(pst[:], nb[:, P * ct : P * (ct + 1)], ident[:])
                    if in_attn:
                        nc.vector.tensor_copy(dstT[ct][:, P * kt : P * (kt + 1)], pst[:])
                    else:
                        trans_copy(dstT[ct][:, P * kt : P * (kt + 1)], pst[:])

            def keepalive(n, uid):
                """Dummy PE transposes — keep the HAM clock up through
                PE-sparse stretches (results unread)."""
                for i in range(n):
                    wps = ps.tile([P, P], BF16, tag="sc", bufs=2, name=f"ka{uid}_{i}")
                    nc.tensor.transpose(wps[:], ident[:], ident[:])

            def emit_ln1(kt, in_attn=False):
                ln_load(kt, xc, "a")
                ln_trans(kt, nT, "a", in_attn)

            wkb_cache = {}

            def load_wk(ot):
                blks = []
                for ci in range(NCT):
                    w = wsp.tile([P, P], BF16, tag="qk", name=f"wk{ot}_{ci}")
                    nc.sync.dma_start(
                        w[:], wk[P * ci : P * (ci + 1), P * ot : P * (ot + 1)]
                    )
                    blks.append(w)
                wkb_cache.clear()
                wkb_cache[ot] = blks

            def emit_kT(ot, tm, in_attn=False):
                if ot not in wkb_cache:
                    load_wk(ot)
                wkb = wkb_cache[ot]
                p = ps_small(f"kps{ot}_{tm}", in_attn=in_attn)
                for ci in range(NCT):
                    nc.tensor.matmul(
                        p[:],
                        wkb[ci][:],
                        nT[ci][:, 512 * tm : 512 * (tm + 1)],
                        start=(ci == 0),
                        stop=(ci == NCT - 1),
                    )
                if in_attn:
                    nc.vector.tensor_copy(kT[ot][:, 512 * tm : 512 * (tm + 1)], p[:])
                else:
                    nc.scalar.activation(
                        kT[ot][:, 512 * tm : 512 * (tm + 1)], p[:], AF.Copy
                    )

            wvhs = {}

            def load_wv(oj):
                wvh = []
                for ci in range(NCT):
                    w = wvp.tile([P, 512], BF16, tag="v", name=f"wv{oj}_{ci}")
                    nc.sync.dma_start(
                        w[:], wv[P * ci : P * (ci + 1), 512 * oj : 512 * (oj + 1)]
                    )
                    wvh.append(w)
                wvhs[oj] = wvh

            def emit_v(oj, tt, in_attn):
                p = ps_small(f"vps{oj}_{tt}", in_attn=in_attn)
                for ci in range(NCT):
                    nc.tensor.matmul(
                        p[:],
                        nT[ci][:, P * tt : P * (tt + 1)],
                        wvhs[oj][ci][:],
                        start=(ci == 0),
                        stop=(ci == NCT - 1),
                    )
                v3 = vA[tt].rearrange("p (h k) -> p h k", k=65)
                ps3 = p.rearrange("p (h k) -> p h k", k=64)
                if in_attn:
                    nc.vector.tensor_copy(v3[:, 8 * oj : 8 * (oj + 1), 0:64], ps3[:])
                else:
                    nc.scalar.activation(
                        v3[:, 8 * oj : 8 * (oj + 1), 0:64], ps3[:], AF.Copy
                    )

            wqb_cache = {}

            def load_wq(ot, m):
                wqb = []
                for ci in range(NCT):
                    w = wsp.tile([P, P], BF16, tag="qk", name=f"wq{ot}_{m}_{ci}")
                    nc.sync.dma_start(
                        w[:], wq[P * ci : P * (ci + 1), P * ot : P * (ot + 1)]
                    )
                    wqb.append(w)
                wqb_cache.clear()
                wqb_cache[ot] = wqb

            def emit_qT(ot, m, in_attn):
                if ot not in wqb_cache:
                    load_wq(ot, m)
                wqb = wqb_cache[ot]
                p = ps_small(f"qps{ot}_{m}", (P, QM), in_attn=in_attn)
                for ci in range(NCT):
                    nc.tensor.matmul(
                        p[:],
                        wqb[ci][:],
                        nqT[ci][:, QM * m : QM * (m + 1)],
                        start=(ci == 0),
                        stop=(ci == NCT - 1),
                    )
                if in_attn:
                    nc.vector.tensor_copy(qT[ot][:, QM * m : QM * (m + 1)], p[:])
                else:
                    nc.scalar.activation(
                        qT[ot][:, QM * m : QM * (m + 1)], p[:], AF.Copy
                    )

            _sc = nc.enter_named_scope("ph_front", False)[0]
            # ---- front (minimal for m=0 start): LN1 tiles 0-7, A2 0-3,
            # kT[*] first-half context, vA0 0-7, qT m=0. The rest streams in
            # as m=0 fillers.
            load_wv(0)
            for kt in range(4):
                emit_ln1(kt)
            emit_kT(0, 0)
            emit_v(0, 0, False)
            emit_v(0, 1, False)
            for kt in range(4, 8):
                emit_ln1(kt)
            emit_kT(0, 1)
            for tt in range(2, 6):
                emit_v(0, tt, False)
            emit_kT(1, 0)
            emit_kT(1, 1)
            emit_v(0, 6, False)
            emit_v(0, 7, False)
            for qt in range(0, 2):
                ln_load(qt, xq, "q")
                ln_trans(qt, nqT, "q")
            for ot in (2, 3):
                emit_kT(ot, 0)
                emit_kT(ot, 1)
            for qt in range(2, 4):
                ln_load(qt, xq, "q")
                ln_trans(qt, nqT, "q")
            for ot in (4, 5, 6, 7):
                emit_kT(ot, 0)
                emit_kT(ot, 1)
            for ot in range(NCT):
                emit_qT(ot, 0, False)

            nc.leave_named_scope("ph_front", _sc, False)

            # ---- attention: m outer, hp inner ------------------------------
            attnT = [
                atp.tile([P, own], BF16, tag="at", name=f"attnT{i}") for i in range(NCT)
            ]
            x2 = [None] * NQT
            wc_blks = {}

            def load_wc(oj, phase):
                blks = []
                for ci in range(NCT):
                    w = wcp.tile([P, 512], BF16, tag="wc", name=f"wc{phase}_{oj}_{ci}")
                    nc.sync.dma_start(
                        w[:], wc[P * ci : P * (ci + 1), 512 * oj : 512 * (oj + 1)]
                    )
                    blks.append(w)
                wc_blks[oj] = blks

            def emit_cproj(qt, oj, in_attn=False):
                if x2[qt] is None:
                    x2[qt] = g3.tile([P, C], FP32, tag="g3", name=f"x2_{qt}")
                xr = exq.tile([P, 512], FP32, tag="xr", bufs=2, name=f"xr{qt}_{oj}")
                nc.sync.dma_start(
                    xr[:], xq[P * qt : P * (qt + 1), 512 * oj : 512 * (oj + 1)]
                )
                p = ps_small(f"cps{qt}_{oj}", in_attn=in_attn)
                for ci in range(NCT):
                    nc.tensor.matmul(
                        p[:],
                        attnT[ci][:, P * qt : P * (qt + 1)],
                        wc_blks[oj][ci][:],
                        start=(ci == 0),
                        stop=(ci == NCT - 1),
                    )
                nc.vector.tensor_add(
                    x2[qt][:, 512 * oj : 512 * (oj + 1)], p[:], xr[:]
                )

            # filler queues, popped INSIDE the j-loop (~2us granularity) so PE
            # duty stays high through scalar-bound attention blocks and the
            # HAM clock never drops. Ordering respects data deps.
            def T(f, *a):
                return lambda: f(*a)

            # NOTE ordering: ALL vA oj=0 tiles must be emitted before
            # load_wv(1) — the wv oj=1 DMAs reuse the wvp ring slots whose
            # release requires every vA0 matmul, and the sync engine is
            # in-order (a late vA0 dep would deadlock the DMA queue).
            # q_m0a must be fully emitted before the hp=4 block of m=0 (its
            # AV matmuls read vA oj=1); q_m0b just needs to land within m=0.
            q_m0a = [
                T(ln_load, 8, xc, "a"), T(ln_load, 9, xc, "a"),
                T(ln_trans, 8, nT, "a", True), T(ln_trans, 9, nT, "a", True),
                T(emit_v, 0, 8, True), T(ln_load, 10, xc, "a"),
                T(emit_v, 0, 9, True), T(ln_load, 11, xc, "a"),
                T(ln_trans, 10, nT, "a", True), T(ln_trans, 11, nT, "a", True),
                T(emit_v, 0, 10, True), T(ln_load, 12, xc, "a"),
                T(emit_v, 0, 11, True), T(ln_load, 13, xc, "a"),
                T(ln_trans, 12, nT, "a", True), T(ln_trans, 13, nT, "a", True),
                T(emit_v, 0, 12, True), T(ln_load, 14, xc, "a"),
                T(emit_v, 0, 13, True), T(ln_load, 15, xc, "a"),
                T(ln_trans, 14, nT, "a", True), T(ln_trans, 15, nT, "a", True),
                T(emit_v, 0, 14, True), T(emit_v, 0, 15, True),
                T(load_wv, 1),
            ]
            for tt in range(NKV):
                q_m0a.append(T(emit_v, 1, tt, True))
            q_m0b = []
            for ot in range(NCT):
                q_m0b.append(T(load_wk, ot))
                q_m0b.append(T(emit_kT, ot, 2, True))
                q_m0b.append(T(emit_kT, ot, 3, True))
            for qt in (4, 5):
                q_m0b.append(T(ln_load, qt, xq, "q"))
            for qt in (4, 5):
                q_m0b.append(T(ln_trans, qt, nqT, "q", True))
            for qt in (6, 7):
                q_m0b.append(T(ln_load, qt, xq, "q"))
            for qt in (6, 7):
                q_m0b.append(T(ln_trans, qt, nqT, "q", True))
            for ot in range(NCT):
                q_m0b.append(T(load_wq, ot, 1))
                q_m0b.append(T(emit_qT, ot, 1, True))
            q_m0 = q_m0a + q_m0b
            q_m0a_set = set(q_m0a)

            q_m1 = [T(load_wc, 0, "m1")]
            for qt in range(4):
                q_m1.append(T(emit_cproj, qt, 0, True))
            q_m1.append(T(load_wc, 1, "m1"))
            for qt in range(4):
                q_m1.append(T(emit_cproj, qt, 1, True))


            mask3 = maskt.rearrange("p (r k) -> p r k", r=2)

            def emit_normalize(hp, r, m, avp_r):
                h = 2 * hp + r
                av_s = smp.tile([65, QM], BF16, tag="avs", bufs=2, name=f"avs{h}_{m}")
                with nc.allow_low_precision(reason="attn out + denom to bf16"):
                    nc.vector.tensor_copy(av_s[:], avp_r[:])
                bcp = ps.tile([64, QM], FP32, tag="bc", bufs=1, name=f"bc{h}_{m}")
                nc.tensor.matmul(
                    bcp[:], ones65[64:65, :], av_s[64:65, :], start=True, stop=True
                )
                bcs = smp.tile([64, QM], FP32, tag="bcs", bufs=2, name=f"bcs{h}_{m}")
                nc.vector.reciprocal_approx_fast(bcs[:], bcp[:])
                nc.vector.tensor_mul(
                    attnT[hp][64 * r : 64 * r + 64, QM * m : QM * (m + 1)],
                    av_s[0:64, :],
                    bcs[:],
                )

            _sc = nc.enter_named_scope("ph_attn", False)[0]
            for m in range(NM):
                jmax0 = 2 * QM * (m + 1) // P
                fq = q_m0 if m == 0 else q_m1
                steps_left = [8 * jmax0]

                def pump():
                    """Pop filler thunks, pacing the queue across the m-block."""
                    if not fq:
                        return
                    n = max(1, -(-len(fq) // max(1, steps_left[0])))
                    for _ in range(min(n, 2)):
                        if fq:
                            fq.pop(0)()
                    steps_left[0] -= 1

                for hp in range(H // 2):
                    if m == 0 and hp == 4:
                        # hp>=4 AV matmuls read vA oj=1 — force q_m0a flushed
                        while fq and fq[0] in q_m0a_set:
                            fq.pop(0)()
                    avp = [
                        ps.tile([65, QM], FP32, tag="av", bufs=2, name=f"av{hp}_{m}_{r}")
                        for r in range(2)
                    ]
                    exs = {}

                    def emit_av(j, r):
                        ex, w0 = exs[(j, r)]
                        nc.tensor.matmul(
                            avp[r][:, w0:QM],
                            vA[j][:, 65 * (2 * hp + r) : 65 * (2 * hp + r) + 65],
                            ex[:, QM * r + w0 : QM * (r + 1)],
                            start=(j == 0),
                            stop=(j == jmax0 - 1),
                        )
                        if r == 1:
                            del exs[(j, 0)], exs[(j, 1)]

                    for j in range(jmax0):
                        wq_ = max(0, (P * j - 2 * QM * m) // 2)
                        sc = ps.tile(
                            [P, 2 * QM], FP32, tag="sc", bufs=2,
                            name=f"sc{hp}_{m}_{j}",
                        )
                        for r in range(2):
                            nc.tensor.matmul(
                                sc[:, QM * r + wq_ : QM * (r + 1)],
                                kT[hp][64 * r : 64 * r + 64, P * j : P * (j + 1)],
                                qT[hp][
                                    64 * r : 64 * r + 64, QM * m + wq_ : QM * (m + 1)
                                ],
                                start=True,
                                stop=True,
                            )
                        ex = exq.tile(
                            [P, 2 * QM], BF16, tag="ex", bufs=3,
                            name=f"ex{hp}_{m}_{j}",
                        )
                        sc3 = sc.rearrange("p (r q) -> p r q", r=2)
                        ex3 = ex.rearrange("p (r q) -> p r q", r=2)
                        nc.scalar.activation(
                            ex3[:, :, wq_:QM], sc3[:, :, wq_:QM], AF.Exp
                        )
                        if P * j >= 2 * QM * m:
                            nc.gpsimd.tensor_mul(
                                ex3[:, :, wq_ : wq_ + 64],
                                ex3[:, :, wq_ : wq_ + 64],
                                mask3[:],
                            )
                        exs[(j, 0)] = (ex, wq_)
                        exs[(j, 1)] = (ex, wq_)
                        if j >= 1:
                            emit_av(j - 1, 0)
                            emit_av(j - 1, 1)
                        if m == 0 or j % 8 == 3:
                            pump()
                    emit_av(jmax0 - 1, 0)
                    emit_av(jmax0 - 1, 1)
                    emit_normalize(hp, 0, m, avp[0])
                    emit_normalize(hp, 1, m, avp[1])
                # everything queued for this m must land before the next m
                while fq:
                    fq.pop(0)()
            nc.leave_named_scope("ph_attn", _sc, False)

            _sc = nc.enter_named_scope("ph_post", False)[0]
            # ---- post: c_proj qt4-7 + LN2 interleaved, MLP in f-quarters ---
            mT = [vap.tile([P, own], BF16, tag="va", name=f"mT{i}") for i in range(NCT)]

            def emit_ln2(qt):
                mb = nbp.tile([P, C], BF16, tag="nb", name=f"mb{qt}")
                layer_norm_to_bf16(x2[qt], mb, f"m{qt}")
                for ct in range(NCT):
                    pst = ps_small(f"mtr{qt}_{ct}", (P, P), BF16)
                    nc.tensor.transpose(pst[:], mb[:, P * ct : P * (ct + 1)], ident[:])
                    trans_copy(mT[ct][:, P * qt : P * (qt + 1)], pst[:])

            def load_wfq(fqi):
                wf4 = []
                for k in range(4):
                    w = g2.tile([P, 2048], BF16, tag="g2", name=f"wf{fqi}_{k}")
                    for half in range(2):
                        ci = 2 * k + half
                        nc.sync.dma_start(
                            w[:, 1024 * half : 1024 * (half + 1)],
                            wf[P * ci : P * (ci + 1), 1024 * fqi : 1024 * (fqi + 1)],
                        )
                    wf4.append(w)
                return wf4

            def load_wpq(fqi):
                wp4 = []
                for k in range(4):
                    w = g2.tile([P, 2048], BF16, tag="g2", name=f"wp{fqi}_{k}")
                    for half in range(2):
                        ftl = 2 * k + half
                        r0 = 1024 * fqi + P * ftl
                        nc.sync.dma_start(
                            w[:, 1024 * half : 1024 * (half + 1)], wp[r0 : r0 + P, :]
                        )
                    wp4.append(w)
                return wp4

            def emit_fc(fqi, wf4, hq, ftl, mq):
                p = ps.tile(
                    [P, QM], FP32, tag="sc", bufs=2, name=f"fps{fqi}_{ftl}_{mq}"
                )
                for ci in range(NCT):
                    nc.tensor.matmul(
                        p[:],
                        wf4[ci // 2][
                            :, 1024 * (ci % 2) + P * ftl : 1024 * (ci % 2)
                            + P * (ftl + 1)
                        ],
                        mT[ci][:, QM * mq : QM * (mq + 1)],
                        start=(ci == 0),
                        stop=(ci == NCT - 1),
                    )
                nc.scalar.activation(
                    hq[ftl][:, QM * mq : QM * (mq + 1)], p[:], AF.Gelu_apprx_tanh
                )

            def emit_proj(fqi, wp4, hq, qt):
                for oj in range(2):
                    p = ps_small(f"pps{fqi}_{qt}_{oj}")
                    for ftl in range(8):
                        nc.tensor.matmul(
                            p[:],
                            hq[ftl][:, P * qt : P * (qt + 1)],
                            wp4[ftl // 2][
                                :, 1024 * (ftl % 2) + 512 * oj : 1024 * (ftl % 2)
                                + 512 * (oj + 1)
                            ],
                            start=(ftl == 0),
                            stop=(ftl == 7),
                        )
                    nc.vector.tensor_add(
                        x2[qt][:, 512 * oj : 512 * (oj + 1)],
                        p[:],
                        x2[qt][:, 512 * oj : 512 * (oj + 1)],
                    )

            # start of post: c_proj(qt4-7) rides over LN2's vector work; the
            # first fc chains overlap LN2 of qt4-7.
            load_wc(0, "p")
            for qt in range(4, 8):
                emit_cproj(qt, 0)
                emit_ln2(qt - 4)
            load_wc(1, "p")
            wf4_0 = load_wfq(0)
            wp4_0 = load_wpq(0)
            for qt in range(4, 8):
                emit_cproj(qt, 1)
            hq0 = [g1.tile([P, own], BF16, tag="g1", name=f"hq0_{i}") for i in range(8)]
            for ftl in range(8):
                emit_fc(0, wf4_0, hq0, ftl, 0)
                if ftl < 4:
                    emit_ln2(4 + ftl)
            for ftl in range(8):
                emit_fc(0, wf4_0, hq0, ftl, 1)
            for qt in range(NQT):
                emit_proj(0, wp4_0, hq0, qt)

            for fqi in range(1, 4):
                wf4 = load_wfq(fqi)
                wp4 = load_wpq(fqi)
                hq = [
                    g1.tile([P, own], BF16, tag="g1", name=f"hq{fqi}_{i}")
                    for i in range(8)
                ]
                for ftl in range(8):
                    for mq in range(2):
                        emit_fc(fqi, wf4, hq, ftl, mq)
                for qt in range(NQT):
                    emit_proj(fqi, wp4, hq, qt)
            for qt in range(NQT):
                nc.sync.dma_start(yout[P * qt : P * (qt + 1), :], x2[qt][:])
            nc.leave_named_scope("ph_post", _sc, False)

    nc.compile()
    return nc


def stage_inputs(x, c_attn_w, c_proj_w, fc_w, proj_w, ln1_g, ln2_g, T=2048, n_cores=8):
    """Host-side prep: per-core input maps. x: (B, T, C) f32."""
    bf = ml_dtypes.bfloat16
    g1w = c_attn_w * ln1_g[:, None]
    wqh = np.ascontiguousarray((g1w[:, 0:C] * 0.125).astype(bf))
    wkh = np.ascontiguousarray(g1w[:, C : 2 * C].astype(bf))
    wvh = np.ascontiguousarray(g1w[:, 2 * C : 3 * C].astype(bf))
    wch = np.ascontiguousarray(c_proj_w.astype(bf))
    wfh = np.ascontiguousarray((fc_w * ln2_g[:, None]).astype(bf))
    wph = np.ascontiguousarray(proj_w.astype(bf))
    in_maps = []
    for c in range(n_cores):
        b, s = c // 2, c % 2
        xcv = np.ascontiguousarray(x[b][:T], dtype=np.float32)
        xqv = np.ascontiguousarray(x[b][s:T:2], dtype=np.float32)
        kvl = np.arange(P)[:, None]
        ul = np.arange(64)[None, :]
        mask = (2 * ul + s >= kvl).astype(np.float32)
        mask = np.tile(mask, (1, 2))
        in_maps.append(
            {
                "xc": xcv,
                "xq": xqv,
                "wq": wqh,
                "wk": wkh,
                "wv": wvh,
                "wc": wch,
                "wf": wfh,
                "wp": wph,
                "msk": mask.astype(bf),
            }
        )
    return in_maps


_NC_CACHE = {}


def _get_nc(T=2048):
    if T not in _NC_CACHE:
        _NC_CACHE[T] = build_nc(T=T)
    return _NC_CACHE[T]


def kernel(**inputs):
    """Full transformer block on 8 NeuronCores. Takes/returns full numpy arrays."""
    from concourse.bass_utils import run_bass_kernel_spmd

    x = np.asarray(inputs["x"], dtype=np.float32)
    B, T, C_ = x.shape
    nc = _get_nc(T=T)
    in_maps = stage_inputs(
        x,
        np.asarray(inputs["c_attn_w"], dtype=np.float32),
        np.asarray(inputs["c_proj_w"], dtype=np.float32),
        np.asarray(inputs["fc_w"], dtype=np.float32),
        np.asarray(inputs["proj_w"], dtype=np.float32),
        np.asarray(inputs["ln1_g"], dtype=np.float32),
        np.asarray(inputs["ln2_g"], dtype=np.float32),
        T=T,
        n_cores=8,
    )
    res = run_bass_kernel_spmd(nc, in_maps, list(range(8)))
    out = np.empty((B, T, C_), dtype=np.float32)
    for c in range(8):
        b, s = c // 2, c % 2
        out[b, s::2, :] = res.results[c]["yout"]
    return out


# revision 53
# speedup vs baseline: 1.1764x; 1.0155x over previous
"""Transformer block (B=4,T=2048,C=1024,H=16) on 8 trn2 cores, zero-communication.

Split: core c -> sequence b=c//2, token parity s=c%2. Each core computes the
full block output for its 1024 strided query tokens (positions s, s+2, ...),
recomputing LN1+K/V for the whole 2048-token context locally (no collectives).

Pipeline (v2): phases are software-pipelined to keep TensorE (and the HAM
clock) saturated end to end:
  front: LN1 tiles interleaved with kT / vA(oj=0) / qT(m=0) projections
  m=0:   attention q-cols 0..511 for all head-pairs; fillers = vA(oj=1),
         qT(m=1)
  m=1:   attention q-cols 512..1023; fillers = c_proj + residual for token
         tiles 0..3
  post:  c_proj qt 4..7, LN2 -> mT, then MLP in 4 f-quarters with
         double-buffered wf/wp streaming; proj accumulates into x2 in SBUF.

PSUM (8 banks): sc 2x[128,1024] (4) | av 2x[*,512] (2) | bc 1 (1) | fill 1 (1).
Softmax normalize: avp -> SBUF early-release copy, denom row broadcast by
matmul, reciprocal_approx_fast, tensor_mul.
"""
import sys

sys.path.insert(0, "/opt/trn_rl_repo")

import numpy as np
import ml_dtypes

import concourse.bass as bass
import concourse.mybir as mybir
import concourse.tile as tile
from concourse import bacc
from concourse.masks import make_identity

FP32 = mybir.dt.float32
BF16 = mybir.dt.bfloat16
AF = mybir.ActivationFunctionType
ALU = mybir.AluOpType

C = 1024
H = 16
HS = 64
FF = 4096
LN_EPS = 1e-5
P = 128


def build_nc(T=2048):
    own = T // 2          # query tokens per core
    NKV = T // P          # kv token tiles (16)
    QM = min(512, own)    # q-macro width
    NM = own // QM        # q macros (2)
    NQT = own // P        # own-token tiles (8)
    NCT = C // P          # feature tiles (8)
    NTM = T // 512        # kv 512-macros (4)

    nc = bacc.Bacc(None, target_bir_lowering=False, debug=False)

    xc = nc.dram_tensor("xc", [T, C], FP32, kind="ExternalInput")
    xq = nc.dram_tensor("xq", [own, C], FP32, kind="ExternalInput")
    wq = nc.dram_tensor("wq", [C, C], BF16, kind="ExternalInput")
    wk = nc.dram_tensor("wk", [C, C], BF16, kind="ExternalInput")
    wv = nc.dram_tensor("wv", [C, C], BF16, kind="ExternalInput")
    wc = nc.dram_tensor("wc", [C, C], BF16, kind="ExternalInput")
    wf = nc.dram_tensor("wf", [C, FF], BF16, kind="ExternalInput")
    wp = nc.dram_tensor("wp", [FF, C], BF16, kind="ExternalInput")
    msk = nc.dram_tensor("msk", [P, 2 * 64], BF16, kind="ExternalInput")
    yout = nc.dram_tensor("yout", [own, C], FP32, kind="ExternalOutput")

    with tile.TileContext(nc) as tc:
        import contextlib

        with contextlib.ExitStack() as ctx:
            const = ctx.enter_context(tc.tile_pool(name="const", bufs=1))
            xin = ctx.enter_context(tc.tile_pool(name="xin", bufs=2))
            lnp = ctx.enter_context(tc.tile_pool(name="lnp", bufs=3))
            nbp = ctx.enter_context(tc.tile_pool(name="nbp", bufs=2))
            g1 = ctx.enter_context(tc.tile_pool(name="g1", bufs=NCT))   # nT -> hq
            g2 = ctx.enter_context(tc.tile_pool(name="g2", bufs=NCT))   # kT -> wf/wp
            g3 = ctx.enter_context(tc.tile_pool(name="g3", bufs=NCT))   # nqT -> x2
            vap = ctx.enter_context(tc.tile_pool(name="vap", bufs=NKV))  # vA -> mT
            qwp = ctx.enter_context(tc.tile_pool(name="qwp", bufs=NCT))  # qT
            wcp = ctx.enter_context(tc.tile_pool(name="wcp", bufs=NCT))  # wc halves
            atp = ctx.enter_context(tc.tile_pool(name="atp", bufs=NCT))  # attnT
            exq = ctx.enter_context(tc.tile_pool(name="exq", bufs=3))    # ex + xr
            smp = ctx.enter_context(tc.tile_pool(name="smp", bufs=3))    # softmax
            wsp = ctx.enter_context(tc.tile_pool(name="wsp", bufs=8))    # wq/wk blk
            wvp = ctx.enter_context(tc.tile_pool(name="wvp", bufs=8))   # wv halves

            ps = ctx.enter_context(tc.tile_pool(name="ps", bufs=1, space="PSUM"))

            _psn = [0, 0]

            def ps_small(name, shape=(P, 512), dtype=FP32, in_attn=False):
                """Rotating one-bank psum tile.

                Outside attention: cycles av/bc/fill (ring of 4 incl. av's 2
                slots). Inside attention the av slots are HELD by the live
                softmax accumulators — allocating av there deadlocks the PE —
                so fillers cycle bc/fill only."""
                if in_attn:
                    tag = ("bc", "fill")[_psn[1] % 2]
                    _psn[1] += 1
                else:
                    tag = ("av", "bc", "fill")[_psn[0] % 3]
                    _psn[0] += 1
                bufs = {"av": 2, "bc": 1, "fill": 1}[tag]
                return ps.tile(list(shape), dtype, tag=tag, bufs=bufs, name=name)

            ident = const.tile([P, P], BF16)
            make_identity(nc, ident)
            ones65 = const.tile([65, 64], BF16)
            nc.vector.memset(ones65[64:65, :], 1.0)

            epst = const.tile([P, 1], FP32)
            nc.vector.memset(epst, LN_EPS)
            maskt = const.tile([P, 2 * 64], BF16)
            nc.sync.dma_start(maskt[:], msk[:])

            # PE warmup: HAM clock ramp while the first LN chunks stream in.
            for wi in range(48):
                wps = ps.tile([P, P], BF16, tag="sc", bufs=2, name=f"warm{wi}")
                nc.tensor.transpose(wps[:], ident[:], ident[:])

            _ceng = [0]

            def trans_copy(dst, src):
                """PSUM->SBUF [128,128] copy, alternated scalar/vector
                (GpSimd cannot read PSUM)."""
                k = _ceng[0] % 2
                _ceng[0] += 1
                if k == 0:
                    nc.scalar.activation(dst, src, AF.Copy)
                else:
                    nc.vector.tensor_copy(dst, src)

            def layer_norm_to_bf16(xt, out_bf, uid):
                """xt [128, C] f32 -> out_bf [128, C] bf16 (normalized).

                Stats on VectorE; the big apply on ScalarE via
                Identity(x*rs + (-mu*rs))."""
                stats = lnp.tile([P, 2, 6], FP32, tag="stats", name=f"st{uid}")
                nc.vector.bn_stats(stats[:, 0, :], xt[:, 0:512])
                nc.vector.bn_stats(stats[:, 1, :], xt[:, 512:1024])
                mv = lnp.tile([P, 2], FP32, tag="mv", name=f"mv{uid}")
                nc.vector.bn_aggr(mv[:], stats[:])
                sd = lnp.tile([P, 1], FP32, tag="sd", name=f"sd{uid}")
                nc.scalar.activation(sd[:], mv[:, 1:2], AF.Sqrt, bias=epst[:])
                rs = lnp.tile([P, 1], FP32, tag="rs", name=f"rs{uid}")
                nc.vector.reciprocal(rs[:], sd[:])
                nmr = lnp.tile([P, 1], FP32, tag="nmr", name=f"nmr{uid}")
                nc.vector.scalar_tensor_tensor(
                    out=nmr[:],
                    in0=mv[:, 0:1],
                    scalar=-1.0,
                    in1=rs[:],
                    op0=ALU.mult,
                    op1=ALU.mult,
                )
                nc.scalar.activation(
                    out_bf[:], xt[:], AF.Identity, bias=nmr[:], scale=rs[:]
                )

            nT = [g1.tile([P, T], BF16, tag="g1", name=f"nT{i}") for i in range(NCT)]
            nqT = [g3.tile([P, own], BF16, tag="g3", name=f"nqT{i}") for i in range(NCT)]
            kT = [g2.tile([P, T], BF16, tag="g2", name=f"kT{i}") for i in range(NCT)]
            qT = [qwp.tile([P, own], BF16, tag="qw", name=f"qT{i}") for i in range(NCT)]
            vA = []
            for tt in range(NKV):
                v = vap.tile([P, H * 65], BF16, tag="va", name=f"vA{tt}")
                v3 = v.rearrange("p (h k) -> p h k", k=65)
                nc.vector.memset(v3[:, :, 64:65], 1.0)
                vA.append(v)

            _ln_nb = {}

            def ln_load(kt, src, pfx):
                """DMA + LN (Vector/Scalar only, no PE) -> staged nb tile."""
                xt = xin.tile([P, C], FP32, tag="xt", name=f"x{pfx}{kt}")
                nc.sync.dma_start(xt[:], src[P * kt : P * (kt + 1), :])
                nb = nbp.tile([P, C], BF16, tag="nb", name=f"nb{pfx}{kt}")
                layer_norm_to_bf16(xt, nb, f"{pfx}{kt}")
                _ln_nb[(pfx, kt)] = nb

            def ln_trans(kt, dstT, pfx, in_attn=False):
                """8 transposes of a staged nb tile into feature-major dstT."""
                nb = _ln_nb.pop((pfx, kt))
                for ct in range(NCT):
                    if in_attn:
                        pst = ps_small(f"tr{pfx}{kt}_{ct}", (P, P), BF16, True)
                    else:
                        pst = ps.tile(
                            [P, P], BF16, tag="sc", bufs=2, name=f"tr{pfx}{kt}_{ct}"
                        )
                    nc.tensor.transpose(pst[:], nb[:, P * ct : P * (ct + 1)], ident[:])
                    if in_attn:
                        nc.vector.tensor_copy(dstT[ct][:, P * kt : P * (kt + 1)], pst[:])
                    else:
                        trans_copy(dstT[ct][:, P * kt : P * (kt + 1)], pst[:])

            def keepalive(n, uid):
                """Dummy PE transposes — keep the HAM clock up through
                PE-sparse stretches (results unread)."""
                for i in range(n):
                    wps = ps.tile([P, P], BF16, tag="sc", bufs=2, name=f"ka{uid}_{i}")
                    nc.tensor.transpose(wps[:], ident[:], ident[:])

            def emit_ln1(kt, in_attn=False):
                ln_load(kt, xc, "a")
                ln_trans(kt, nT, "a", in_attn)

            wkb_cache = {}

            def load_wk(ot):
                blks = []
                for ci in range(NCT):
                    w = wsp.tile([P, P], BF16, tag="qk", name=f"wk{ot}_{ci}")
                    nc.sync.dma_start(
                        w[:], wk[P * ci : P * (ci + 1), P * ot : P * (ot + 1)]
                    )
                    blks.append(w)
                wkb_cache.clear()
                wkb_cache[ot] = blks

            def emit_kT(ot, tm, in_attn=False):
                if ot not in wkb_cache:
                    load_wk(ot)
                wkb = wkb_cache[ot]
                p = ps_small(f"kps{ot}_{tm}", in_attn=in_attn)
                for ci in range(NCT):
                    nc.tensor.matmul(
                        p[:],
                        wkb[ci][:],
                        nT[ci][:, 512 * tm : 512 * (tm + 1)],
                        start=(ci == 0),
                        stop=(ci == NCT - 1),
                    )
                if in_attn:
                    nc.vector.tensor_copy(kT[ot][:, 512 * tm : 512 * (tm + 1)], p[:])
                else:
                    nc.scalar.activation(
                        kT[ot][:, 512 * tm : 512 * (tm + 1)], p[:], AF.Copy
                    )

            wvhs = {}

            def load_wv(oj):
                wvh = []
                for ci in range(NCT):
                    w = wvp.tile([P, 512], BF16, tag="v", name=f"wv{oj}_{ci}")
                    nc.sync.dma_start(
                        w[:], wv[P * ci : P * (ci + 1), 512 * oj : 512 * (oj + 1)]
                    )
                    wvh.append(w)
                wvhs[oj] = wvh

            def emit_v(oj, tt, in_attn):
                p = ps_small(f"vps{oj}_{tt}", in_attn=in_attn)
                for ci in range(NCT):
                    nc.tensor.matmul(
                        p[:],
                        nT[ci][:, P * tt : P * (tt + 1)],
                        wvhs[oj][ci][:],
                        start=(ci == 0),
                        stop=(ci == NCT - 1),
                    )
                v3 = vA[tt].rearrange("p (h k) -> p h k", k=65)
                ps3 = p.rearrange("p (h k) -> p h k", k=64)
                if in_attn:
                    nc.vector.tensor_copy(v3[:, 8 * oj : 8 * (oj + 1), 0:64], ps3[:])
                else:
                    nc.scalar.activation(
                        v3[:, 8 * oj : 8 * (oj + 1), 0:64], ps3[:], AF.Copy
                    )

            wqb_cache = {}

            def load_wq(ot, m):
                wqb = []
                for ci in range(NCT):
                    w = wsp.tile([P, P], BF16, tag="qk", name=f"wq{ot}_{m}_{ci}")
                    nc.sync.dma_start(
                        w[:], wq[P * ci : P * (ci + 1), P * ot : P * (ot + 1)]
                    )
                    wqb.append(w)
                wqb_cache.clear()
                wqb_cache[ot] = wqb

            def emit_qT(ot, m, in_attn):
                if ot not in wqb_cache:
                    load_wq(ot, m)
                wqb = wqb_cache[ot]
                p = ps_small(f"qps{ot}_{m}", (P, QM), in_attn=in_attn)
                for ci in range(NCT):
                    nc.tensor.matmul(
                        p[:],
                        wqb[ci][:],
                        nqT[ci][:, QM * m : QM * (m + 1)],
                        start=(ci == 0),
                        stop=(ci == NCT - 1),
                    )
                if in_attn:
                    nc.vector.tensor_copy(qT[ot][:, QM * m : QM * (m + 1)], p[:])
                else:
                    nc.scalar.activation(
                        qT[ot][:, QM * m : QM * (m + 1)], p[:], AF.Copy
                    )

            _sc = nc.enter_named_scope("ph_front", False)[0]
            # ---- front (minimal for m=0 start): LN1 tiles 0-7, A2 0-3,
            # kT[*] first-half context, vA0 0-7, qT m=0. The rest streams in
            # as m=0 fillers.
            load_wv(0)
            for kt in range(4):
                emit_ln1(kt)
            emit_kT(0, 0)
            emit_v(0, 0, False)
            emit_v(0, 1, False)
            for kt in range(4, 8):
                emit_ln1(kt)
            emit_kT(0, 1)
            for tt in range(2, 6):
                emit_v(0, tt, False)
            emit_kT(1, 0)
            emit_kT(1, 1)
            emit_v(0, 6, False)
            emit_v(0, 7, False)
            for qt in range(0, 2):
                ln_load(qt, xq, "q")
                ln_trans(qt, nqT, "q")
            for ot in (2, 3):
                emit_kT(ot, 0)
                emit_kT(ot, 1)
            for qt in range(2, 4):
                ln_load(qt, xq, "q")
                ln_trans(qt, nqT, "q")
            for ot in (4, 5, 6, 7):
                emit_kT(ot, 0)
                emit_kT(ot, 1)
            for ot in range(NCT):
                emit_qT(ot, 0, False)

            nc.leave_named_scope("ph_front", _sc, False)

            # ---- attention: m outer, hp inner ------------------------------
            attnT = [
                atp.tile([P, own], BF16, tag="at", name=f"attnT{i}") for i in range(NCT)
            ]
            x2 = [None] * NQT
            wc_blks = {}

            def load_wc(oj, phase):
                blks = []
                for ci in range(NCT):
                    w = wcp.tile([P, 512], BF16, tag="wc", name=f"wc{phase}_{oj}_{ci}")
                    nc.sync.dma_start(
                        w[:], wc[P * ci : P * (ci + 1), 512 * oj : 512 * (oj + 1)]
                    )
                    blks.append(w)
                wc_blks[oj] = blks

            def emit_cproj(qt, oj, in_attn=False):
                if x2[qt] is None:
                    x2[qt] = g3.tile([P, C], FP32, tag="g3", name=f"x2_{qt}")
                xr = exq.tile([P, 512], FP32, tag="xr", bufs=2, name=f"xr{qt}_{oj}")
                nc.sync.dma_start(
                    xr[:], xq[P * qt : P * (qt + 1), 512 * oj : 512 * (oj + 1)]
                )
                p = ps_small(f"cps{qt}_{oj}", in_attn=in_attn)
                for ci in range(NCT):
                    nc.tensor.matmul(
                        p[:],
                        attnT[ci][:, P * qt : P * (qt + 1)],
                        wc_blks[oj][ci][:],
                        start=(ci == 0),
                        stop=(ci == NCT - 1),
                    )
                nc.vector.tensor_add(
                    x2[qt][:, 512 * oj : 512 * (oj + 1)], p[:], xr[:]
                )

            # filler queues, popped INSIDE the j-loop (~2us granularity) so PE
            # duty stays high through scalar-bound attention blocks and the
            # HAM clock never drops. Ordering respects data deps.
            def T(f, *a):
                return lambda: f(*a)

            # NOTE ordering: ALL vA oj=0 tiles must be emitted before
            # load_wv(1) — the wv oj=1 DMAs reuse the wvp ring slots whose
            # release requires every vA0 matmul, and the sync engine is
            # in-order (a late vA0 dep would deadlock the DMA queue).
            # q_m0a must be fully emitted before the hp=4 block of m=0 (its
            # AV matmuls read vA oj=1); q_m0b just needs to land within m=0.
            q_m0a = [
                T(ln_load, 8, xc, "a"), T(ln_load, 9, xc, "a"),
                T(ln_trans, 8, nT, "a", True), T(ln_trans, 9, nT, "a", True),
                T(emit_v, 0, 8, True), T(ln_load, 10, xc, "a"),
                T(emit_v, 0, 9, True), T(ln_load, 11, xc, "a"),
                T(ln_trans, 10, nT, "a", True), T(ln_trans, 11, nT, "a", True),
                T(emit_v, 0, 10, True), T(ln_load, 12, xc, "a"),
                T(emit_v, 0, 11, True), T(ln_load, 13, xc, "a"),
                T(ln_trans, 12, nT, "a", True), T(ln_trans, 13, nT, "a", True),
                T(emit_v, 0, 12, True), T(ln_load, 14, xc, "a"),
                T(emit_v, 0, 13, True), T(ln_load, 15, xc, "a"),
                T(ln_trans, 14, nT, "a", True), T(ln_trans, 15, nT, "a", True),
                T(emit_v, 0, 14, True), T(emit_v, 0, 15, True),
                T(load_wv, 1),
            ]
            for tt in range(NKV):
                q_m0a.append(T(emit_v, 1, tt, True))
            q_m0b = []
            for ot in range(NCT):
                q_m0b.append(T(load_wk, ot))
                q_m0b.append(T(emit_kT, ot, 2, True))
                q_m0b.append(T(emit_kT, ot, 3, True))
            for qt in (4, 5):
                q_m0b.append(T(ln_load, qt, xq, "q"))
            for qt in (4, 5):
                q_m0b.append(T(ln_trans, qt, nqT, "q", True))
            for qt in (6, 7):
                q_m0b.append(T(ln_load, qt, xq, "q"))
            for qt in (6, 7):
                q_m0b.append(T(ln_trans, qt, nqT, "q", True))
            for ot in range(NCT):
                q_m0b.append(T(load_wq, ot, 1))
                q_m0b.append(T(emit_qT, ot, 1, True))
            q_m0 = q_m0a + q_m0b
            q_m0a_set = set(q_m0a)

            q_m1 = [T(load_wc, 0, "m1")]
            for qt in range(4):
                q_m1.append(T(emit_cproj, qt, 0, True))
            q_m1.append(T(load_wc, 1, "m1"))
            for qt in range(4):
                q_m1.append(T(emit_cproj, qt, 1, True))


            mask3 = maskt.rearrange("p (r k) -> p r k", r=2)

            def emit_normalize(hp, r, m, avp_r):
                h = 2 * hp + r
                av_s = smp.tile([65, QM], BF16, tag="avs", bufs=2, name=f"avs{h}_{m}")
                with nc.allow_low_precision(reason="attn out + denom to bf16"):
                    nc.vector.tensor_copy(av_s[:], avp_r[:])
                bcp = ps.tile([64, QM], FP32, tag="bc", bufs=1, name=f"bc{h}_{m}")
                nc.tensor.matmul(
                    bcp[:], ones65[64:65, :], av_s[64:65, :], start=True, stop=True
                )
                bcs = smp.tile([64, QM], FP32, tag="bcs", bufs=2, name=f"bcs{h}_{m}")
                nc.vector.reciprocal_approx_fast(bcs[:], bcp[:])
                nc.vector.tensor_mul(
                    attnT[hp][64 * r : 64 * r + 64, QM * m : QM * (m + 1)],
                    av_s[0:64, :],
                    bcs[:],
                )

            _sc = nc.enter_named_scope("ph_attn", False)[0]
            for m in range(NM):
                jmax0 = 2 * QM * (m + 1) // P
                fq = q_m0 if m == 0 else q_m1
                steps_left = [8 * jmax0]
                # m=1: pump fires twice per block (j=3, j=11) -> 16 calls.
                # Defer pops so the last filler lands in the LAST block —
                # late-m1 blocks otherwise run dry and drop the HAM clock.
                pump_calls_left = [16]

                def pump():
                    """Pop filler thunks, pacing the queue across the m-block."""
                    if m == 1:
                        if fq and len(fq) >= pump_calls_left[0]:
                            fq.pop(0)()
                        pump_calls_left[0] -= 1
                        return
                    if not fq:
                        return
                    n = max(1, -(-len(fq) // max(1, steps_left[0])))
                    for _ in range(min(n, 2)):
                        if fq:
                            fq.pop(0)()
                    steps_left[0] -= 1

                for hp in range(H // 2):
                    if m == 0 and hp == 4:
                        # hp>=4 AV matmuls read vA oj=1 — force q_m0a flushed
                        while fq and fq[0] in q_m0a_set:
                            fq.pop(0)()
                    avp = [
                        ps.tile([65, QM], FP32, tag="av", bufs=2, name=f"av{hp}_{m}_{r}")
                        for r in range(2)
                    ]
                    exs = {}

                    def emit_av(j, r):
                        ex, w0 = exs[(j, r)]
                        nc.tensor.matmul(
                            avp[r][:, w0:QM],
                            vA[j][:, 65 * (2 * hp + r) : 65 * (2 * hp + r) + 65],
                            ex[:, QM * r + w0 : QM * (r + 1)],
                            start=(j == 0),
                            stop=(j == jmax0 - 1),
                        )
                        if r == 1:
                            del exs[(j, 0)], exs[(j, 1)]

                    for j in range(jmax0):
                        wq_ = max(0, (P * j - 2 * QM * m) // 2)
                        sc = ps.tile(
                            [P, 2 * QM], FP32, tag="sc", bufs=2,
                            name=f"sc{hp}_{m}_{j}",
                        )
                        for r in range(2):
                            nc.tensor.matmul(
                                sc[:, QM * r + wq_ : QM * (r + 1)],
                                kT[hp][64 * r : 64 * r + 64, P * j : P * (j + 1)],
                                qT[hp][
                                    64 * r : 64 * r + 64, QM * m + wq_ : QM * (m + 1)
                                ],
                                start=True,
                                stop=True,
                            )
                        ex = exq.tile(
                            [P, 2 * QM], BF16, tag="ex", bufs=3,
                            name=f"ex{hp}_{m}_{j}",
                        )
                        sc3 = sc.rearrange("p (r q) -> p r q", r=2)
                        ex3 = ex.rearrange("p (r q) -> p r q", r=2)
                        nc.scalar.activation(
                            ex3[:, :, wq_:QM], sc3[:, :, wq_:QM], AF.Exp
                        )
                        if P * j >= 2 * QM * m:
                            nc.gpsimd.tensor_mul(
                                ex3[:, :, wq_ : wq_ + 64],
                                ex3[:, :, wq_ : wq_ + 64],
                                mask3[:],
                            )
                        exs[(j, 0)] = (ex, wq_)
                        exs[(j, 1)] = (ex, wq_)
                        if j >= 1:
                            emit_av(j - 1, 0)
                            emit_av(j - 1, 1)
                        if m == 0 or j % 8 == 3:
                            pump()
                    emit_av(jmax0 - 1, 0)
                    emit_av(jmax0 - 1, 1)
                    emit_normalize(hp, 0, m, avp[0])
                    emit_normalize(hp, 1, m, avp[1])
                # everything queued for this m must land before the next m
                while fq:
                    fq.pop(0)()
            nc.leave_named_scope("ph_attn", _sc, False)

            _sc = nc.enter_named_scope("ph_post", False)[0]
            # ---- post: c_proj qt4-7 + LN2 interleaved, MLP in f-quarters ---
            mT = [vap.tile([P, own], BF16, tag="va", name=f"mT{i}") for i in range(NCT)]

            def emit_ln2(qt):
                mb = nbp.tile([P, C], BF16, tag="nb", name=f"mb{qt}")
                layer_norm_to_bf16(x2[qt], mb, f"m{qt}")
                for ct in range(NCT):
                    pst = ps_small(f"mtr{qt}_{ct}", (P, P), BF16)
                    nc.tensor.transpose(pst[:], mb[:, P * ct : P * (ct + 1)], ident[:])
                    trans_copy(mT[ct][:, P * qt : P * (qt + 1)], pst[:])

            def load_wfq(fqi):
                wf4 = []
                for k in range(4):
                    w = g2.tile([P, 2048], BF16, tag="g2", name=f"wf{fqi}_{k}")
                    for half in range(2):
                        ci = 2 * k + half
                        nc.sync.dma_start(
                            w[:, 1024 * half : 1024 * (half + 1)],
                            wf[P * ci : P * (ci + 1), 1024 * fqi : 1024 * (fqi + 1)],
                        )
                    wf4.append(w)
                return wf4

            def load_wpq(fqi):
                wp4 = []
                for k in range(4):
                    w = g2.tile([P, 2048], BF16, tag="g2", name=f"wp{fqi}_{k}")
                    for half in range(2):
                        ftl = 2 * k + half
                        r0 = 1024 * fqi + P * ftl
                        nc.sync.dma_start(
                            w[:, 1024 * half : 1024 * (half + 1)], wp[r0 : r0 + P, :]
                        )
                    wp4.append(w)
                return wp4

            def emit_fc(fqi, wf4, hq, ftl, mq):
                p = ps.tile(
                    [P, QM], FP32, tag="sc", bufs=2, name=f"fps{fqi}_{ftl}_{mq}"
                )
                for ci in range(NCT):
                    nc.tensor.matmul(
                        p[:],
                        wf4[ci // 2][
                            :, 1024 * (ci % 2) + P * ftl : 1024 * (ci % 2)
                            + P * (ftl + 1)
                        ],
                        mT[ci][:, QM * mq : QM * (mq + 1)],
                        start=(ci == 0),
                        stop=(ci == NCT - 1),
                    )
                nc.scalar.activation(
                    hq[ftl][:, QM * mq : QM * (mq + 1)], p[:], AF.Gelu_apprx_tanh
                )

            def emit_proj(fqi, wp4, hq, qt):
                for oj in range(2):
                    p = ps_small(f"pps{fqi}_{qt}_{oj}")
                    for ftl in range(8):
                        nc.tensor.matmul(
                            p[:],
                            hq[ftl][:, P * qt : P * (qt + 1)],
                            wp4[ftl // 2][
                                :, 1024 * (ftl % 2) + 512 * oj : 1024 * (ftl % 2)
                                + 512 * (oj + 1)
                            ],
                            start=(ftl == 0),
                            stop=(ftl == 7),
                        )
                    nc.vector.tensor_add(
                        x2[qt][:, 512 * oj : 512 * (oj + 1)],
                        p[:],
                        x2[qt][:, 512 * oj : 512 * (oj + 1)],
                    )

            # start of post: c_proj(qt4-7) rides over LN2's vector work; the
            # first fc chains overlap LN2 of qt4-7.
            load_wc(0, "p")
            for qt in range(4, 8):
                emit_cproj(qt, 0)
                emit_ln2(qt - 4)
            load_wc(1, "p")
            wf4_0 = load_wfq(0)
            wp4_0 = load_wpq(0)
            for qt in range(4, 8):
                emit_cproj(qt, 1)
            hq0 = [g1.tile([P, own], BF16, tag="g1", name=f"hq0_{i}") for i in range(8)]
            for ftl in range(8):
                emit_fc(0, wf4_0, hq0, ftl, 0)
                if ftl < 4:
                    emit_ln2(4 + ftl)
            for ftl in range(8):
                emit_fc(0, wf4_0, hq0, ftl, 1)
            for qt in range(NQT):
                emit_proj(0, wp4_0, hq0, qt)

            for fqi in range(1, 4):
                wf4 = load_wfq(fqi)
                wp4 = load_wpq(fqi)
                hq = [
                    g1.tile([P, own], BF16, tag="g1", name=f"hq{fqi}_{i}")
                    for i in range(8)
                ]
                for ftl in range(8):
                    for mq in range(2):
                        emit_fc(fqi, wf4, hq, ftl, mq)
                for qt in range(NQT):
                    emit_proj(fqi, wp4, hq, qt)
            for qt in range(NQT):
                nc.sync.dma_start(yout[P * qt : P * (qt + 1), :], x2[qt][:])
            nc.leave_named_scope("ph_post", _sc, False)

    nc.compile()
    return nc


def stage_inputs(x, c_attn_w, c_proj_w, fc_w, proj_w, ln1_g, ln2_g, T=2048, n_cores=8):
    """Host-side prep: per-core input maps. x: (B, T, C) f32."""
    bf = ml_dtypes.bfloat16
    g1w = c_attn_w * ln1_g[:, None]
    wqh = np.ascontiguousarray((g1w[:, 0:C] * 0.125).astype(bf))
    wkh = np.ascontiguousarray(g1w[:, C : 2 * C].astype(bf))
    wvh = np.ascontiguousarray(g1w[:, 2 * C : 3 * C].astype(bf))
    wch = np.ascontiguousarray(c_proj_w.astype(bf))
    wfh = np.ascontiguousarray((fc_w * ln2_g[:, None]).astype(bf))
    wph = np.ascontiguousarray(proj_w.astype(bf))
    in_maps = []
    for c in range(n_cores):
        b, s = c // 2, c % 2
        xcv = np.ascontiguousarray(x[b][:T], dtype=np.float32)
        xqv = np.ascontiguousarray(x[b][s:T:2], dtype=np.float32)
        kvl = np.arange(P)[:, None]
        ul = np.arange(64)[None, :]
        mask = (2 * ul + s >= kvl).astype(np.float32)
        mask = np.tile(mask, (1, 2))
        in_maps.append(
            {
                "xc": xcv,
                "xq": xqv,
                "wq": wqh,
                "wk": wkh,
                "wv": wvh,
                "wc": wch,
                "wf": wfh,
                "wp": wph,
                "msk": mask.astype(bf),
            }
        )
    return in_maps


_NC_CACHE = {}


def _get_nc(T=2048):
    if T not in _NC_CACHE:
        _NC_CACHE[T] = build_nc(T=T)
    return _NC_CACHE[T]


def kernel(**inputs):
    """Full transformer block on 8 NeuronCores. Takes/returns full numpy arrays."""
    from concourse.bass_utils import run_bass_kernel_spmd

    x = np.asarray(inputs["x"], dtype=np.float32)
    B, T, C_ = x.shape
    nc = _get_nc(T=T)
    in_maps = stage_inputs(
        x,
        np.asarray(inputs["c_attn_w"], dtype=np.float32),
        np.asarray(inputs["c_proj_w"], dtype=np.float32),
        np.asarray(inputs["fc_w"], dtype=np.float32),
        np.asarray(inputs["proj_w"], dtype=np.float32),
        np.asarray(inputs["ln1_g"], dtype=np.float32),
        np.asarray(inputs["ln2_g"], dtype=np.float32),
        T=T,
        n_cores=8,
    )
    res = run_bass_kernel_spmd(nc, in_maps, list(range(8)))
    out = np.empty((B, T, C_), dtype=np.float32)
    for c in range(8):
        b, s = c // 2, c % 2
        out[b, s::2, :] = res.results[c]["yout"]
    return out
